# revision 1
# baseline (speedup 1.0000x reference)
"""JointAttention TRN2 Bass kernel.

Sharding: 8 cores = batch(2) x head-group(4). Each core owns one batch
element and 4 of the 16 heads (a 256-wide channel slice). Per core:
  qT/kT projections in [c, t] layout (lhsT = W natural, rhs = xT moving),
  v projection in [t, c] layout (lhsT = xT chunk stationary, rhs = W),
  scores^T = K^T.T @ Q^T per 128-key chunk ([k, q] layout, row-tiled
  2 heads at a time), exp on ScalarE, PV with V-augmented-ones columns
  giving the softmax denominators, division via a ones-matmul broadcast,
  and the output projection (row-parallel Wo slice). The 4 partial
  outputs per batch element are summed on the host (row-parallel
  all-reduce as part of unsharding) and bo is added once.
"""

import sys

import numpy as np

if "/opt/trn_rl_repo" not in sys.path:
    sys.path.insert(0, "/opt/trn_rl_repo")

import concourse.bass as bass
import concourse.tile as tile
from concourse import bacc, mybir
from concourse.bass_utils import run_bass_kernel_spmd

F32 = mybir.dt.float32
AFT = mybir.ActivationFunctionType

D = 1024          # model dim
T = 2048          # query length (= self key length)
TK = 4096         # total key length (self + context)
CS = 256          # channels per core (4 heads x 64)
NH = 4            # heads per core
HD = 64           # head dim
DC = 8            # D chunks of 128
N_CORES = 8

# Storage dtype for every tile that feeds the PE array: float32r runs the
# matmul at 1 cyc/row (vs 4 for float32) with reduced-precision multiplies.
# The BIR verifier requires producers of fp32r matmul operands to emit
# fp32r themselves, so these tiles are allocated as float32r and DRAM-side
# DMA access patterns are bitcast views.
MDT = mybir.dt.float32r


def _dr(ap):
    return ap.bitcast(MDT) if MDT is not F32 else ap


def build_nc():
    nc = bacc.Bacc(None)

    xT = nc.declare_dram_parameter("xT", [D, T], F32, isOutput=False)
    cT = nc.declare_dram_parameter("cT", [D, T], F32, isOutput=False)
    wq = nc.declare_dram_parameter("wq", [D, CS], F32, isOutput=False)
    wks = nc.declare_dram_parameter("wks", [D, CS], F32, isOutput=False)
    wkc = nc.declare_dram_parameter("wkc", [D, CS], F32, isOutput=False)
    wvs = nc.declare_dram_parameter("wvs", [D, CS], F32, isOutput=False)
    wvc = nc.declare_dram_parameter("wvc", [D, CS], F32, isOutput=False)
    bq = nc.declare_dram_parameter("bq", [CS, 1], F32, isOutput=False)
    bks = nc.declare_dram_parameter("bks", [CS, 1], F32, isOutput=False)
    bkc = nc.declare_dram_parameter("bkc", [CS, 1], F32, isOutput=False)
    bvs = nc.declare_dram_parameter("bvs", [1, CS], F32, isOutput=False)
    bvc = nc.declare_dram_parameter("bvc", [1, CS], F32, isOutput=False)
    wo = nc.declare_dram_parameter("wo", [CS, D], F32, isOutput=False)
    ones64 = nc.declare_dram_parameter("ones64", [1, HD], F32, isOutput=False)
    out = nc.declare_dram_parameter("out", [T, D], F32, isOutput=True)

    with tile.TileContext(nc) as tc:
        _emit(nc, tc, xT, cT, wq, wks, wkc, wvs, wvc,
              bq, bks, bkc, bvs, bvc, wo, ones64, out)
    nc.compile()
    return nc


def _emit(nc, tc, xT, cT, wq, wks, wkc, wvs, wvc, bq, bks, bkc, bvs, bvc,
          wo, ones64, out):
    from contextlib import ExitStack

    ctx = ExitStack()
    with ctx:
        consts = ctx.enter_context(tc.tile_pool(name="consts", bufs=1))
        w_rot = ctx.enter_context(tc.tile_pool(name="w_rot", bufs=1))
        io_pool = ctx.enter_context(tc.tile_pool(name="io", bufs=16))
        qt_pool = ctx.enter_context(tc.tile_pool(name="qt", bufs=1))
        kt_pool = ctx.enter_context(tc.tile_pool(name="kt", bufs=1))
        v_pool = ctx.enter_context(tc.tile_pool(name="v", bufs=1))
        p_pool = ctx.enter_context(tc.tile_pool(name="p", bufs=4))
        outt_pool = ctx.enter_context(tc.tile_pool(name="outt", bufs=1))
        stage_pool = ctx.enter_context(tc.tile_pool(name="stage", bufs=3))
        misc_pool = ctx.enter_context(tc.tile_pool(name="misc", bufs=2))
        # PSUM: shared(2) + scores(4) + pv(2) = 8 banks
        ps_shared = ctx.enter_context(
            tc.tile_pool(name="ps_shared", bufs=2, space="PSUM"))
        ps_scores = ctx.enter_context(
            tc.tile_pool(name="ps_scores", bufs=2, space="PSUM"))
        ps_pv = ctx.enter_context(
            tc.tile_pool(name="ps_pv", bufs=2, space="PSUM"))

        # ---- constants ----
        w_sb = {}
        t = consts.tile([128, DC, CS], MDT, tag="w_wq", name="w_wq")
        nc.sync.dma_start(out=t, in_=_dr(wq.rearrange("(a p) c -> p a c", p=128)))
        w_sb["wq"] = t
        b_sb = {}
        for name, b in (("bq", bq), ("bks", bks), ("bkc", bkc)):
            t = consts.tile([128, 2], F32, tag=f"b_{name}", name=f"b_{name}")
            nc.sync.dma_start(out=t, in_=b.rearrange("(a p) o -> p (a o)", p=128))
            b_sb[name] = t
        bv_sb = {}
        for name, b in (("bvs", bvs), ("bvc", bvc)):
            t = consts.tile([128, CS], F32, tag=f"bv_{name}", name=f"bv_{name}")
            nc.sync.dma_start(out=t, in_=b[:, :].to_broadcast([128, CS]))
            bv_sb[name] = t
        wo_sb = consts.tile([128, 2, D], MDT, tag="wo")
        nc.sync.dma_start(out=wo_sb, in_=_dr(wo.rearrange("(a p) f -> p a f", p=128)))
        ones_sb = consts.tile([128, HD], MDT, tag="ones")
        nc.sync.dma_start(out=ones_sb,
                          in_=_dr(ones64[:, :].to_broadcast([128, HD])))

        # ---- projections ----
        qT_sb = [qt_pool.tile([128, T], MDT, tag=f"qT{cc}", name=f"qT{cc}")
                 for cc in range(2)]
        kT_sb = [kt_pool.tile([128, TK], MDT, tag=f"kT{cc}", name=f"kT{cc}")
                 for cc in range(2)]
        v_sb = [v_pool.tile([128, NH * (HD + 1)], MDT, tag=f"v{kc}", name=f"v{kc}")
                for kc in range(32)]

        for src_i, (src, wk_n, wv_n, bk_n, bv_n) in enumerate((
                (xT, "wks", "wvs", "bks", "bvs"),
                (cT, "wkc", "wvc", "bkc", "bvc"))):
            # rotating weight tiles (self weights, then ctx weights)
            for name, w, wtag in ((wk_n, (wks, wkc)[src_i], "wk"),
                                  (wv_n, (wvs, wvc)[src_i], "wv")):
                t = w_rot.tile([128, DC, CS], MDT, tag=wtag, name=f"w_{name}")
                nc.sync.dma_start(out=t, in_=_dr(w.rearrange("(a p) c -> p a c", p=128)))
                w_sb[name] = t
            # projections, emitted wave-major (one 512-wide t-column of the
            # source at a time) so io pool slots rotate without cycles
            projs = [(wk_n, bk_n, kT_sb, src_i * T)]
            if src_i == 0:
                projs.append(("wq", "bq", qT_sb, 0))
            for tc4 in range(4):
                wave = []
                for dc in range(DC):
                    t = io_pool.tile([128, 512], MDT, tag="io",
                                     name=f"io_{src_i}_{tc4}_{dc}")
                    nc.sync.dma_start(
                        out=t,
                        in_=_dr(src[dc * 128:(dc + 1) * 128,
                                    tc4 * 512:(tc4 + 1) * 512]))
                    wave.append(t)

                # q^T / k^T projections: [c, t] layout
                for wn, bn, dst, coff in projs:
                    for cc in range(2):
                        ps = ps_shared.tile([128, 512], F32, tag="ps")
                        for dc in range(DC):
                            nc.tensor.matmul(
                                ps,
                                (w_sb[wn][:, dc, cc * 128:(cc + 1) * 128]),
                                (wave[dc]),
                                start=(dc == 0), stop=(dc == DC - 1))
                        nc.vector.tensor_scalar_add(
                            dst[cc][:, coff + tc4 * 512:coff + (tc4 + 1) * 512],
                            ps, b_sb[bn][:, cc:cc + 1])

                # v projection: [t, c] layout, per 128-key chunk
                for sub in range(4):
                    tc_i = tc4 * 4 + sub
                    ps = ps_shared.tile([128, 512], F32, tag="ps")
                    for dc in range(DC):
                        nc.tensor.matmul(
                            ps[:, 0:CS],
                            (wave[dc][:, sub * 128:(sub + 1) * 128]),
                            (w_sb[wv_n][:, dc, :]),
                            start=(dc == 0), stop=(dc == DC - 1))
                    vt = v_sb[src_i * 16 + tc_i]
                    for h in range(NH):
                        nc.vector.tensor_add(
                            vt[:, h * 65:h * 65 + 64],
                            ps[:, h * 64:(h + 1) * 64],
                            bv_sb[bv_n][:, h * 64:(h + 1) * 64])
                    vt_ones = vt[:].rearrange(
                        "p (h x) -> p h x", h=NH)[:, :, 64:65].rearrange(
                        "p h one -> p (h one)")
                    nc.sync.dma_start(
                        out=vt_ones,
                        in_=_dr(ones64[:, 0:NH].to_broadcast([128, NH])))

        # ---- attention ----
        outT_sb = [outt_pool.tile([128, T], MDT, tag=f"outT{cc}", name=f"outT{cc}")
                   for cc in range(2)]
        for qc in range(4):
            qs = slice(qc * 512, (qc + 1) * 512)
            for pair in range(2):
                hA, hB = 2 * pair, 2 * pair + 1
                pvA = ps_pv.tile([128, 512], F32, tag="pv")
                pvB = ps_pv.tile([128, 512], F32, tag="pv")
                for kc in range(32):
                    ks = slice(kc * 128, (kc + 1) * 128)
                    s2 = ps_scores.tile([128, 1024], F32, tag="s")
                    nc.tensor.matmul(
                        s2[:, 0:512], (kT_sb[pair][0:64, ks]),
                        (qT_sb[pair][0:64, qs]), start=True, stop=True)
                    nc.tensor.matmul(
                        s2[:, 512:1024], (kT_sb[pair][64:128, ks]),
                        (qT_sb[pair][64:128, qs]), start=True, stop=True,
                        tile_position=(64, 0))
                    pt = p_pool.tile([128, 1024], MDT, tag="pt")
                    nc.scalar.activation(pt, s2, AFT.Exp)
                    st, sp = kc == 0, kc == 31
                    vt = v_sb[kc]
                    # [V | ones] stationary: rows 0-63 = head out^T,
                    # row 64 = softmax denominator
                    nc.tensor.matmul(
                        pvA[0:65, :], vt[:, hA * 65:(hA + 1) * 65],
                        pt[:, 0:512], start=st, stop=sp)
                    nc.tensor.matmul(
                        pvB[0:65, :], vt[:, hB * 65:(hB + 1) * 65],
                        pt[:, 512:1024], start=st, stop=sp)
                # softmax division epilogue; the reciprocal rows sit at
                # partition 64 (where the PV denominators land)
                rt = misc_pool.tile([128, 1024], MDT, tag="recip")
                with nc.allow_low_precision(
                        reason="fp32r rounding of softmax reciprocal"):
                    nc.vector.reciprocal(rt[64:65, 0:512], pvA[64:65, :])
                    nc.vector.reciprocal(rt[64:65, 512:1024], pvB[64:65, :])
                pvs = misc_pool.tile([128, 1024], F32, tag="pvs")
                nc.vector.tensor_copy(pvs[0:64, 0:512], pvA[0:64, :])
                nc.vector.tensor_copy(pvs[0:64, 512:1024], pvB[0:64, :])
                bcA = ps_shared.tile([128, 512], F32, tag="ps")
                nc.tensor.matmul(
                    bcA[0:64, :], ones_sb[64:65, 0:64], rt[64:65, 0:512],
                    start=True, stop=True, tile_position=(64, 0))
                nc.vector.tensor_mul(
                    outT_sb[pair][0:64, qs], pvs[0:64, 0:512], bcA[0:64, :])
                bcB = ps_shared.tile([128, 512], F32, tag="ps")
                nc.tensor.matmul(
                    bcB[0:64, :], ones_sb[64:65, 0:64], rt[64:65, 512:1024],
                    start=True, stop=True, tile_position=(64, 0))
                odd = misc_pool.tile([128, 512], MDT, tag="odd")
                nc.vector.tensor_mul(
                    odd[0:64, :], pvs[0:64, 512:1024], bcB[0:64, :])
                # odd head lives at partitions 64-127 of outT: shift via DMA
                nc.sync.dma_start(out=outT_sb[pair][64:128, qs],
                                  in_=odd[0:64, :])

        # ---- output projection (partial; host sums across head groups) ----
        for qt in range(16):
            qsl = slice(qt * 128, (qt + 1) * 128)
            for fc in range(2):
                fsl = slice(fc * 512, (fc + 1) * 512)
                ps = ps_scores.tile([128, 512], F32, tag="s")
                for cc in range(2):
                    nc.tensor.matmul(
                        ps, (outT_sb[cc][:, qsl]), (wo_sb[:, cc, fsl]),
                        start=(cc == 0), stop=(cc == 1))
                st = stage_pool.tile([128, 512], F32, tag="stage")
                nc.vector.tensor_copy(st, ps)
                nc.sync.dma_start(out=out[qsl, fsl], in_=st)


_NC_CACHE = None


def kernel(**inputs):
    global _NC_CACHE
    if _NC_CACHE is None:
        _NC_CACHE = build_nc()
    nc = _NC_CACHE

    f = {k: np.asarray(v, dtype=np.float32) for k, v in inputs.items()}
    x, context = f["x"], f["context"]
    B = x.shape[0]

    xTs = [np.ascontiguousarray(x[b].T) for b in range(B)]
    cTs = [np.ascontiguousarray(context[b].T) for b in range(B)]

    in_maps = []
    for b in range(B):
        for hg in range(4):
            sl = slice(hg * CS, (hg + 1) * CS)
            in_maps.append({
                "xT": xTs[b],
                "cT": cTs[b],
                "wq": np.ascontiguousarray(f["Wq"][:, sl]) * 0.125,
                "wks": np.ascontiguousarray(f["Wks"][:, sl]),
                "wkc": np.ascontiguousarray(f["Wkc"][:, sl]),
                "wvs": np.ascontiguousarray(f["Wvs"][:, sl]),
                "wvc": np.ascontiguousarray(f["Wvc"][:, sl]),
                "bq": (f["bq"][sl] * 0.125).reshape(CS, 1).copy(),
                "bks": f["bks"][sl].reshape(CS, 1).copy(),
                "bkc": f["bkc"][sl].reshape(CS, 1).copy(),
                "bvs": f["bvs"][sl].reshape(1, CS).copy(),
                "bvc": f["bvc"][sl].reshape(1, CS).copy(),
                "wo": np.ascontiguousarray(f["Wo"][sl, :]),
                "ones64": np.ones((1, HD), dtype=np.float32),
            })

    res = run_bass_kernel_spmd(nc, in_maps, list(range(N_CORES))).results

    bo = f["bo"]
    out = np.empty((B, T, D), dtype=np.float32)
    for b in range(B):
        acc = res[b * 4 + 0]["out"].astype(np.float32).copy()
        for hg in range(1, 4):
            acc += res[b * 4 + hg]["out"]
        out[b] = acc + bo
    return out



# revision 4
# speedup vs baseline: 1.0593x; 1.0593x over previous
"""JointAttention TRN2 Bass kernel.

Sharding: 8 cores = batch(2) x head-group(4). Each core owns one batch
element and 4 of the 16 heads (a 256-wide channel slice). All matmul
operands are bf16 (1 cyc/row on the PE at any free size); accumulation
stays fp32 in PSUM.

Per core:
  qT/kT projections in [c, t] layout (lhsT = W stationary, rhs = xT
  moving), v projection in [t, c] layout (lhsT = xT chunk stationary,
  rhs = W moving), scores^T = K^T.T @ Q^T per 128-key chunk ([k, q]
  layout, 2 heads row-tiled via tile_position), exp on ScalarE
  (activation engine is the critical resource: ~1.04us per [128,1024]
  tile), PV with V-augmented-ones columns giving the softmax
  denominators, division via a ones-matmul broadcast, and the output
  projection (row-parallel Wo slice).

Scheduling: the attention inner loop is software-pipelined (PV lags QK
by one k-chunk so the PE never waits on the exp) and all epilogues +
the output projection are emitted through a pending-work queue that
drips them into the PE slack of subsequent k-iterations, keeping the
activation engine saturated. The 4 partial outputs per batch element
are summed on the host (row-parallel all-reduce as part of unsharding)
and bo is added once.
"""

import sys
from collections import deque

import numpy as np

if "/opt/trn_rl_repo" not in sys.path:
    sys.path.insert(0, "/opt/trn_rl_repo")

import ml_dtypes

import concourse.bass as bass
import concourse.tile as tile
from concourse import bacc, mybir
from concourse.bass_utils import run_bass_kernel_spmd

F32 = mybir.dt.float32
BF16 = mybir.dt.bfloat16
F32R = mybir.dt.float32r
AFT = mybir.ActivationFunctionType

D = 1024          # model dim
T = 2048          # query length (= self key length)
TK = 4096         # total key length (self + context)
CS = 256          # channels per core (4 heads x 64)
NH = 4            # heads per core
HD = 64           # head dim
DC = 8            # D chunks of 128
N_CORES = 8

BF = ml_dtypes.bfloat16


def build_nc():
    nc = bacc.Bacc(None)

    xT = nc.declare_dram_parameter("xT", [D, T], BF16, isOutput=False)
    cT = nc.declare_dram_parameter("cT", [D, T], BF16, isOutput=False)
    wq = nc.declare_dram_parameter("wq", [D, CS], BF16, isOutput=False)
    wks = nc.declare_dram_parameter("wks", [D, CS], BF16, isOutput=False)
    wkc = nc.declare_dram_parameter("wkc", [D, CS], BF16, isOutput=False)
    wvs = nc.declare_dram_parameter("wvs", [D, CS], BF16, isOutput=False)
    wvc = nc.declare_dram_parameter("wvc", [D, CS], BF16, isOutput=False)
    bq = nc.declare_dram_parameter("bq", [CS, 1], F32, isOutput=False)
    bks = nc.declare_dram_parameter("bks", [CS, 1], F32, isOutput=False)
    bkc = nc.declare_dram_parameter("bkc", [CS, 1], F32, isOutput=False)
    bvs = nc.declare_dram_parameter("bvs", [1, CS], F32, isOutput=False)
    bvc = nc.declare_dram_parameter("bvc", [1, CS], F32, isOutput=False)
    wo = nc.declare_dram_parameter("wo", [CS, D], BF16, isOutput=False)
    out = nc.declare_dram_parameter("out", [T, D], BF16, isOutput=True)

    with tile.TileContext(nc) as tc:
        _emit(nc, tc, xT, cT, wq, wks, wkc, wvs, wvc,
              bq, bks, bkc, bvs, bvc, wo, out)
    nc.compile()
    return nc


def _emit(nc, tc, xT, cT, wq, wks, wkc, wvs, wvc, bq, bks, bkc, bvs, bvc,
          wo, out):
    from contextlib import ExitStack

    ctx = ExitStack()
    with ctx:
        consts = ctx.enter_context(tc.tile_pool(name="consts", bufs=1))
        w_rot = ctx.enter_context(tc.tile_pool(name="w_rot", bufs=1))
        io_pool = ctx.enter_context(tc.tile_pool(name="io", bufs=16))
        qt_pool = ctx.enter_context(tc.tile_pool(name="qt", bufs=1))
        kt_pool = ctx.enter_context(tc.tile_pool(name="kt", bufs=1))
        v_pool = ctx.enter_context(tc.tile_pool(name="v", bufs=1))
        p_pool = ctx.enter_context(tc.tile_pool(name="p", bufs=4))
        outt_pool = ctx.enter_context(tc.tile_pool(name="outt", bufs=1))
        stage_pool = ctx.enter_context(tc.tile_pool(name="stage", bufs=3))
        misc_pool = ctx.enter_context(tc.tile_pool(name="misc", bufs=2))
        # PSUM: shared(2) + scores(4) + pv(2) = 8 banks
        ps_shared = ctx.enter_context(
            tc.tile_pool(name="ps_shared", bufs=2, space="PSUM"))
        ps_scores = ctx.enter_context(
            tc.tile_pool(name="ps_scores", bufs=2, space="PSUM"))
        ps_pv = ctx.enter_context(
            tc.tile_pool(name="ps_pv", bufs=2, space="PSUM"))

        # ---- constants (DMA order matters: wq + biases first; wo later) ----
        w_sb = {}
        t = consts.tile([128, DC, CS], BF16, tag="w_wq", name="w_wq")
        nc.sync.dma_start(out=t, in_=wq.rearrange("(a p) c -> p a c", p=128))
        w_sb["wq"] = t
        b_sb = {}
        for name, b in (("bq", bq), ("bks", bks), ("bkc", bkc)):
            t = consts.tile([128, 2], F32, tag=f"b_{name}", name=f"b_{name}")
            nc.sync.dma_start(out=t, in_=b.rearrange("(a p) o -> p (a o)", p=128))
            b_sb[name] = t
        bv_sb = {}
        for name, b in (("bvs", bvs), ("bvc", bvc)):
            t = consts.tile([128, CS], F32, tag=f"bv_{name}", name=f"bv_{name}")
            nc.sync.dma_start(out=t, in_=b[:, :].to_broadcast([128, CS]))
            bv_sb[name] = t
        # ones rows for the softmax-denominator broadcast matmul: built with
        # a memset (no DRAM traffic)
        ones_sb = consts.tile([128, HD], BF16, tag="ones")
        nc.vector.memset(ones_sb[:], 1.0)
        wo_sb = consts.tile([128, 2, D], BF16, tag="wo")

        # ---- projections ----
        qT_sb = [qt_pool.tile([128, T], BF16, tag=f"qT{cc}", name=f"qT{cc}")
                 for cc in range(2)]
        kT_sb = [kt_pool.tile([128, TK], BF16, tag=f"kT{cc}", name=f"kT{cc}")
                 for cc in range(2)]
        v_sb = [v_pool.tile([128, NH * (HD + 1)], BF16, tag=f"v{kc}",
                            name=f"v{kc}")
                for kc in range(32)]

        for src_i, (src, wk_n, wv_n, bk_n, bv_n) in enumerate((
                (xT, "wks", "wvs", "bks", "bvs"),
                (cT, "wkc", "wvc", "bkc", "bvc"))):
            # rotating weight tiles (self weights, then ctx weights)
            for name, w, wtag in ((wk_n, (wks, wkc)[src_i], "wk"),
                                  (wv_n, (wvs, wvc)[src_i], "wv")):
                t = w_rot.tile([128, DC, CS], BF16, tag=wtag, name=f"w_{name}")
                nc.sync.dma_start(out=t, in_=w.rearrange("(a p) c -> p a c", p=128))
                w_sb[name] = t
            # projections, emitted wave-major (one 512-wide t-column of the
            # source at a time) so io pool slots rotate without cycles
            projs = [(wk_n, bk_n, kT_sb, src_i * T)]
            if src_i == 0:
                projs.append(("wq", "bq", qT_sb, 0))
            for tc4 in range(4):
                wave = []
                for dc in range(DC):
                    t = io_pool.tile([128, 512], BF16, tag="io",
                                     name=f"io_{src_i}_{tc4}_{dc}")
                    nc.sync.dma_start(
                        out=t,
                        in_=src[dc * 128:(dc + 1) * 128,
                                tc4 * 512:(tc4 + 1) * 512])
                    wave.append(t)
                if src_i == 0 and tc4 == 0:
                    # wo is first needed by the interleaved out-projection,
                    # well into the attention phase; emit its DMA after the
                    # first wave so it doesn't delay the first matmuls
                    nc.sync.dma_start(
                        out=wo_sb, in_=wo.rearrange("(a p) f -> p a f", p=128))

                # q^T / k^T projections: [c, t] layout
                for wn, bn, dst, coff in projs:
                    for cc in range(2):
                        ps = ps_shared.tile([128, 512], F32, tag="ps")
                        for dc in range(DC):
                            nc.tensor.matmul(
                                ps,
                                (w_sb[wn][:, dc, cc * 128:(cc + 1) * 128]),
                                (wave[dc]),
                                start=(dc == 0), stop=(dc == DC - 1))
                        nc.vector.tensor_scalar_add(
                            dst[cc][:, coff + tc4 * 512:coff + (tc4 + 1) * 512],
                            ps, b_sb[bn][:, cc:cc + 1])

                # v projection: [t, c] layout, per 128-key chunk
                for sub in range(4):
                    tc_i = tc4 * 4 + sub
                    ps = ps_shared.tile([128, 512], F32, tag="ps")
                    for dc in range(DC):
                        nc.tensor.matmul(
                            ps[:, 0:CS],
                            (wave[dc][:, sub * 128:(sub + 1) * 128]),
                            (w_sb[wv_n][:, dc, :]),
                            start=(dc == 0), stop=(dc == DC - 1))
                    vt = v_sb[src_i * 16 + tc_i]
                    vt_v = vt[:].rearrange("p (h x) -> p h x", h=NH)
                    nc.vector.tensor_add(
                        vt_v[:, :, 0:HD],
                        ps[:, 0:CS].rearrange("p (h x) -> p h x", h=NH),
                        bv_sb[bv_n][:, :].rearrange("p (h x) -> p h x", h=NH))
                    nc.vector.memset(
                        vt_v[:, :, HD:HD + 1].rearrange("p h one -> p (h one)"),
                        1.0)

        # ---- attention ----
        # pending: deque of closures, each emitting ~200-500ns of PE work
        # (plus any amount of off-PE work).  Popped one per k-iteration to
        # fill the PE slack under the activation-bound steady state.
        pending = deque()

        outT_sb = [outt_pool.tile([128, T], BF16, tag=f"outT{cc}",
                                  name=f"outT{cc}")
                   for cc in range(2)]

        def make_epilogue(pair, qs, pvA, pvB):
            # softmax division epilogue; the reciprocal rows sit at
            # partition 64 (where the PV denominators land)
            def epiA():
                rt = misc_pool.tile([128, 1024], BF16, tag="recip")
                with nc.allow_low_precision(
                        reason="bf16 rounding of softmax reciprocal"):
                    nc.vector.reciprocal(rt[64:65, 0:512], pvA[64:65, :])
                pvs = misc_pool.tile([128, 512], F32, tag="pvsA")
                nc.vector.tensor_copy(pvs[0:64, :], pvA[0:64, :])
                bcA = ps_shared.tile([128, 512], F32, tag="ps")
                nc.tensor.matmul(
                    bcA[0:64, :], ones_sb[64:65, 0:64], rt[64:65, 0:512],
                    start=True, stop=True, tile_position=(64, 0))
                nc.vector.tensor_mul(
                    outT_sb[pair][0:64, qs], pvs[0:64, :], bcA[0:64, :])
                epilogue_state["rt"] = rt

            def epiB():
                rt = epilogue_state["rt"]
                with nc.allow_low_precision(
                        reason="bf16 rounding of softmax reciprocal"):
                    nc.vector.reciprocal(rt[64:65, 512:1024], pvB[64:65, :])
                pvs = misc_pool.tile([128, 512], F32, tag="pvsB")
                nc.vector.tensor_copy(pvs[0:64, :], pvB[0:64, :])
                bcB = ps_shared.tile([128, 512], F32, tag="ps")
                nc.tensor.matmul(
                    bcB[0:64, :], ones_sb[64:65, 0:64], rt[64:65, 512:1024],
                    start=True, stop=True, tile_position=(64, 0))
                odd = misc_pool.tile([128, 512], BF16, tag="odd")
                nc.vector.tensor_mul(
                    odd[0:64, :], pvs[0:64, :], bcB[0:64, :])
                # odd head lives at partitions 64-127 of outT: shift via DMA
                nc.sync.dma_start(out=outT_sb[pair][64:128, qs],
                                  in_=odd[0:64, :])

            return [epiA, epiB]

        epilogue_state = {}

        def make_outproj(qc):
            # output projection for one 512-query block (row-parallel Wo
            # slice; host sums partials across head groups)
            work = []
            for qt in range(qc * 4, qc * 4 + 4):
                qsl = slice(qt * 128, (qt + 1) * 128)
                for fc in range(2):
                    fsl = slice(fc * 512, (fc + 1) * 512)

                    def mm(qsl=qsl, fsl=fsl):
                        ps = ps_shared.tile([128, 512], F32, tag="ps")
                        for cc in range(2):
                            nc.tensor.matmul(
                                ps, (outT_sb[cc][:, qsl]),
                                (wo_sb[:, cc, fsl]),
                                start=(cc == 0), stop=(cc == 1))
                        st = stage_pool.tile([128, 512], BF16, tag="stage")
                        nc.vector.tensor_copy(st, ps)
                        nc.sync.dma_start(out=out[qsl, fsl], in_=st)

                    work.append(mm)
            return work

        def emit_pv(pvA, pvB, hA, hB, pt, kc):
            # [V | ones] stationary: rows 0-63 = head out^T, row 64 =
            # softmax denominator
            vt = v_sb[kc]
            nc.tensor.matmul(
                pvA[0:65, :], vt[:, hA * 65:(hA + 1) * 65],
                pt[:, 0:512], start=(kc == 0), stop=(kc == 31))
            nc.tensor.matmul(
                pvB[0:65, :], vt[:, hB * 65:(hB + 1) * 65],
                pt[:, 512:1024], start=(kc == 0), stop=(kc == 31))

        for qc in range(4):
            qs = slice(qc * 512, (qc + 1) * 512)
            for pair in range(2):
                hA, hB = 2 * pair, 2 * pair + 1
                pvA = ps_pv.tile([128, 512], F32, tag="pv")
                pvB = ps_pv.tile([128, 512], F32, tag="pv")
                prev_pt = None
                for kc in range(32):
                    ks = slice(kc * 128, (kc + 1) * 128)
                    s2 = ps_scores.tile([128, 1024], F32, tag="s")
                    nc.tensor.matmul(
                        s2[:, 0:512], (kT_sb[pair][0:64, ks]),
                        (qT_sb[pair][0:64, qs]), start=True, stop=True)
                    nc.tensor.matmul(
                        s2[:, 512:1024], (kT_sb[pair][64:128, ks]),
                        (qT_sb[pair][64:128, qs]), start=True, stop=True,
                        tile_position=(64, 0))
                    pt = p_pool.tile([128, 1024], BF16, tag="pt")
                    nc.scalar.activation(pt, s2, AFT.Exp)
                    # drip one queued closure into this iteration's PE slack
                    # (popped before emit_pv so a new block's first PV
                    # matmuls are emitted after the previous block's
                    # epilogue reads of the same PSUM slots)
                    if pending:
                        pending.popleft()()
                    # PV lags one k-chunk so the PE never stalls on the exp
                    if prev_pt is not None:
                        emit_pv(pvA, pvB, hA, hB, prev_pt, kc - 1)
                    prev_pt = pt
                # flush last k-chunk of this block
                emit_pv(pvA, pvB, hA, hB, prev_pt, 31)
                pending.extend(make_epilogue(pair, qs, pvA, pvB))
            pending.extend(make_outproj(qc))
        while pending:
            pending.popleft()()


_NC_CACHE = None


def kernel(**inputs):
    global _NC_CACHE
    if _NC_CACHE is None:
        _NC_CACHE = build_nc()
    nc = _NC_CACHE

    f = {k: np.asarray(v, dtype=np.float32) for k, v in inputs.items()}
    x, context = f["x"], f["context"]
    B = x.shape[0]

    xTs = [np.ascontiguousarray(x[b].T).astype(BF) for b in range(B)]
    cTs = [np.ascontiguousarray(context[b].T).astype(BF) for b in range(B)]

    in_maps = []
    for b in range(B):
        for hg in range(4):
            sl = slice(hg * CS, (hg + 1) * CS)
            in_maps.append({
                "xT": xTs[b],
                "cT": cTs[b],
                "wq": (np.ascontiguousarray(f["Wq"][:, sl]) * 0.125).astype(BF),
                "wks": np.ascontiguousarray(f["Wks"][:, sl]).astype(BF),
                "wkc": np.ascontiguousarray(f["Wkc"][:, sl]).astype(BF),
                "wvs": np.ascontiguousarray(f["Wvs"][:, sl]).astype(BF),
                "wvc": np.ascontiguousarray(f["Wvc"][:, sl]).astype(BF),
                "bq": (f["bq"][sl] * 0.125).reshape(CS, 1).copy(),
                "bks": f["bks"][sl].reshape(CS, 1).copy(),
                "bkc": f["bkc"][sl].reshape(CS, 1).copy(),
                "bvs": f["bvs"][sl].reshape(1, CS).copy(),
                "bvc": f["bvc"][sl].reshape(1, CS).copy(),
                "wo": np.ascontiguousarray(f["Wo"][sl, :]).astype(BF),
            })

    res = run_bass_kernel_spmd(nc, in_maps, list(range(N_CORES))).results

    bo = f["bo"]
    out = np.empty((B, T, D), dtype=np.float32)
    for b in range(B):
        acc = res[b * 4 + 0]["out"].astype(np.float32)
        for hg in range(1, 4):
            acc += res[b * 4 + hg]["out"].astype(np.float32)
        out[b] = acc + bo
    return out


# revision 9
# speedup vs baseline: 1.0929x; 1.0317x over previous
"""JointAttention TRN2 Bass kernel.

Sharding: 8 cores = batch(2) x head-group(4). Each core owns one batch
element and 4 of the 16 heads (a 256-wide channel slice). All matmul
operands are bf16 (1 cyc/row on the PE at any free size); accumulation
stays fp32 in PSUM.

Per core:
  qT/kT projections in [c, t] layout (lhsT = W stationary, rhs = xT
  moving), v projection in [t, c] layout (lhsT = xT chunk stationary,
  rhs = W moving), scores^T = K^T.T @ Q^T per 128-key chunk ([k, q]
  layout, 2 heads row-tiled via tile_position), exp on ScalarE
  (activation engine is the critical resource: ~1.04us per [128,1024]
  tile), PV with V-augmented-ones columns giving the softmax
  denominators, division via a ones-matmul broadcast, and the output
  projection (row-parallel Wo slice).

Scheduling: everything except the QK->exp->PV spine is emitted through
a deadline-driven work queue that drips projection chains, softmax
epilogues and the output projection into the PE slack of the attention
k-iterations, so the activation engine starts exp-ing within a few us
of t=0 and never starves. The attention spine itself is software-
pipelined (PV lags QK by one k-chunk). The 4 partial outputs per batch
element are summed on the host (row-parallel all-reduce as part of
unsharding) and bo is added once.
"""

import sys
from collections import defaultdict, deque

import numpy as np

if "/opt/trn_rl_repo" not in sys.path:
    sys.path.insert(0, "/opt/trn_rl_repo")

import ml_dtypes

import concourse.bass as bass
import concourse.tile as tile
from concourse import bacc, mybir
from concourse.bass_utils import run_bass_kernel_spmd

F32 = mybir.dt.float32
BF16 = mybir.dt.bfloat16
AFT = mybir.ActivationFunctionType

D = 1024          # model dim
T = 2048          # query length (= self key length)
TK = 4096         # total key length (self + context)
CS = 256          # channels per core (4 heads x 64)
NH = 4            # heads per core
HD = 64           # head dim
DC = 8            # D chunks of 128
N_CORES = 8

BF = ml_dtypes.bfloat16


def build_nc():
    nc = bacc.Bacc(None)

    xT = nc.declare_dram_parameter("xT", [D, T], BF16, isOutput=False)
    cT = nc.declare_dram_parameter("cT", [D, T], BF16, isOutput=False)
    wq = nc.declare_dram_parameter("wq", [D, CS], BF16, isOutput=False)
    wks = nc.declare_dram_parameter("wks", [D, CS], BF16, isOutput=False)
    wkc = nc.declare_dram_parameter("wkc", [D, CS], BF16, isOutput=False)
    wvs = nc.declare_dram_parameter("wvs", [D, CS], BF16, isOutput=False)
    wvc = nc.declare_dram_parameter("wvc", [D, CS], BF16, isOutput=False)
    bq = nc.declare_dram_parameter("bq", [CS, 1], F32, isOutput=False)
    bks = nc.declare_dram_parameter("bks", [CS, 1], F32, isOutput=False)
    bkc = nc.declare_dram_parameter("bkc", [CS, 1], F32, isOutput=False)
    bvs = nc.declare_dram_parameter("bvs", [1, CS], F32, isOutput=False)
    bvc = nc.declare_dram_parameter("bvc", [1, CS], F32, isOutput=False)
    wo = nc.declare_dram_parameter("wo", [CS, D], BF16, isOutput=False)
    out = nc.declare_dram_parameter("out", [T, D], BF16, isOutput=True)

    with tile.TileContext(nc) as tc:
        _emit(nc, tc, xT, cT, wq, wks, wkc, wvs, wvc,
              bq, bks, bkc, bvs, bvc, wo, out)
    nc.compile()
    return nc


def _emit(nc, tc, xT, cT, wq, wks, wkc, wvs, wvc, bq, bks, bkc, bvs, bvc,
          wo, out):
    from contextlib import ExitStack

    ctx = ExitStack()
    with ctx:
        consts = ctx.enter_context(tc.tile_pool(name="consts", bufs=1))
        wpool = ctx.enter_context(tc.tile_pool(name="wpool", bufs=1))
        io_pool = ctx.enter_context(tc.tile_pool(name="io", bufs=64))
        qt_pool = ctx.enter_context(tc.tile_pool(name="qt", bufs=1))
        kt_pool = ctx.enter_context(tc.tile_pool(name="kt", bufs=1))
        v_pool = ctx.enter_context(tc.tile_pool(name="v", bufs=1))
        p_pool = ctx.enter_context(tc.tile_pool(name="p", bufs=4))
        outt_pool = ctx.enter_context(tc.tile_pool(name="outt", bufs=1))
        stage_pool = ctx.enter_context(tc.tile_pool(name="stage", bufs=3))
        misc_pool = ctx.enter_context(tc.tile_pool(name="misc", bufs=2))
        # PSUM: shared(2) + scores(4) + pv(2) = 8 banks
        ps_shared = ctx.enter_context(
            tc.tile_pool(name="ps_shared", bufs=2, space="PSUM"))
        ps_scores = ctx.enter_context(
            tc.tile_pool(name="ps_scores", bufs=2, space="PSUM"))
        ps_pv = ctx.enter_context(
            tc.tile_pool(name="ps_pv", bufs=2, space="PSUM"))

        # ---- small constants (cheap DMAs, emitted up front) ----
        b_sb = {}
        for name, b in (("bq", bq), ("bks", bks), ("bkc", bkc)):
            t = consts.tile([128, 2], F32, tag=f"b_{name}", name=f"b_{name}")
            nc.sync.dma_start(out=t, in_=b.rearrange("(a p) o -> p (a o)", p=128))
            b_sb[name] = t
        bv_sb = {}
        for name, b in (("bvs", bvs), ("bvc", bvc)):
            t = consts.tile([128, CS], F32, tag=f"bv_{name}", name=f"bv_{name}")
            nc.sync.dma_start(out=t, in_=b[:, :].to_broadcast([128, CS]))
            bv_sb[name] = t
        ones_sb = consts.tile([128, HD], BF16, tag="ones")
        nc.vector.memset(ones_sb[:], 1.0)

        # weight tiles: each gets its own slot (bf16 keeps SBUF cheap)
        w_sb = {}
        for name, w in (("wq", wq), ("wks", wks), ("wvs", wvs),
                        ("wkc", wkc), ("wvc", wvc)):
            w_sb[name] = wpool.tile([128, DC, CS], BF16, tag=f"w_{name}",
                                    name=f"w_{name}")
        wo_sb = consts.tile([128, 2, D], BF16, tag="wo")

        qT_sb = [qt_pool.tile([128, T], BF16, tag=f"qT{cc}", name=f"qT{cc}")
                 for cc in range(2)]
        kT_sb = [kt_pool.tile([128, TK], BF16, tag=f"kT{cc}", name=f"kT{cc}")
                 for cc in range(2)]
        v_sb = [v_pool.tile([128, NH * (HD + 1)], BF16, tag=f"v{kc}",
                            name=f"v{kc}")
                for kc in range(32)]
        outT_sb = [outt_pool.tile([128, T], BF16, tag=f"outT{cc}",
                                  name=f"outT{cc}")
                   for cc in range(2)]

        # ---- deadline-scheduled work ----------------------------------
        # sched[(bi, kc)] = closures that MUST be emitted right after the
        # exp of iteration (block bi, k-chunk kc); kc == -1 means at block
        # start, before its first QK.  `pending` holds order-only work
        # (epilogues, out-projection) popped one per iteration when no
        # deadline work is due.
        sched = defaultdict(list)
        pending = deque()

        srcs = ((xT, "wks", "wvs", "bks", "bvs"),
                (cT, "wkc", "wvc", "bkc", "bvc"))
        waves = [[None] * DC for _ in range(8)]  # 8 waves of 8 [128,512] tiles
        chain_ps = {}

        def io_dma(w):
            src = srcs[w // 4][0]
            tc4 = w % 4

            def go():
                for dc in range(DC):
                    t = io_pool.tile([128, 512], BF16, tag="io",
                                     name=f"io_{w}_{dc}")
                    nc.sync.dma_start(
                        out=t,
                        in_=src[dc * 128:(dc + 1) * 128,
                                tc4 * 512:(tc4 + 1) * 512])
                    waves[w][dc] = t

            return go

        def w_dma(name, w):
            def go():
                nc.sync.dma_start(
                    out=w_sb[name], in_=w.rearrange("(a p) c -> p a c", p=128))

            return go

        def wo_dma():
            nc.sync.dma_start(
                out=wo_sb, in_=wo.rearrange("(a p) f -> p a f", p=128))

        def ckproj(w, cc, half, dst, coff, wn, bn):
            # half a [c,t]-projection chain (4 of 8 contraction steps);
            # halves share one PSUM accumulation group via chain_ps
            tc4 = w % 4

            def go():
                key = ("ck", w, cc, wn)
                if half == 0:
                    chain_ps[key] = ps_shared.tile(
                        [128, 512], F32, tag="ps", name=f"ps_ck_{w}_{cc}_{wn}")
                ps = chain_ps[key]
                for dc in range(half * 4, half * 4 + 4):
                    nc.tensor.matmul(
                        ps, (w_sb[wn][:, dc, cc * 128:(cc + 1) * 128]),
                        (waves[w][dc]),
                        start=(dc == 0), stop=(dc == DC - 1))
                if half == 1:
                    nc.vector.tensor_scalar_add(
                        dst[cc][:, coff + tc4 * 512:coff + (tc4 + 1) * 512],
                        ps, b_sb[bn][:, cc:cc + 1])

            return go

        def vproj(w, sub, pair, half, wv_n, bv_n):
            # half a [t,c]-projection chain for one head pair (128 cols)
            kc = (w // 4) * 16 + (w % 4) * 4 + sub

            def go():
                key = ("v", w, sub, pair)
                if half == 0:
                    chain_ps[key] = ps_shared.tile(
                        [128, 512], F32, tag="ps", name=f"ps_v_{w}_{sub}_{pair}")
                ps = chain_ps[key]
                for dc in range(half * 4, half * 4 + 4):
                    nc.tensor.matmul(
                        ps[:, 0:128],
                        (waves[w][dc][:, sub * 128:(sub + 1) * 128]),
                        (w_sb[wv_n][:, dc, pair * 128:(pair + 1) * 128]),
                        start=(dc == 0), stop=(dc == DC - 1))
                if half == 1:
                    vt = v_sb[kc]
                    vt_v = vt[:].rearrange("p (h x) -> p h x", h=NH)
                    nc.vector.tensor_add(
                        vt_v[:, 2 * pair:2 * pair + 2, 0:HD],
                        ps[:, 0:128].rearrange("p (h x) -> p h x", h=2),
                        bv_sb[bv_n][:, pair * 128:(pair + 1) * 128]
                        .rearrange("p (h x) -> p h x", h=2))
                    nc.vector.memset(
                        vt_v[:, 2 * pair:2 * pair + 2, HD:HD + 1]
                        .rearrange("p h one -> p (h one)"), 1.0)

            return go

        # deadline assignment. blocks: bi = qc*2 + pair, 32 k-iterations
        # each.  An item due at (bi, kc) is emitted after exp(kc) and
        # before PV(kc-1) of that iteration.
        for w in range(8):
            src, wk_n, wv_n, bk_n, bv_n = srcs[w // 4]
            kc0 = (w // 4) * 16 + (w % 4) * 4  # first k-chunk of this wave
            # io prefetch ~6 iterations ahead of first use
            sched[(0, max(kc0 - 7, -1) if w else -1)].append(io_dma(w))
            # kT chains: cc=0 feeds block 0, cc=1 feeds block 1
            for cc in range(2):
                bi = cc
                due = max(kc0 - 2, -1) if (w or cc) else -1
                sched[(bi, due)].append(
                    ckproj(w, cc, 0, kT_sb, (w // 4) * T, wk_n, bk_n))
                sched[(bi, max(kc0 - 1, -1) if (w or cc) else -1)].append(
                    ckproj(w, cc, 1, kT_sb, (w // 4) * T, wk_n, bk_n))
            # v chains: pair p feeds block p, chunk kc0+sub needed by
            # PV at iteration kc0+sub+1
            for sub in range(4):
                for pair in range(2):
                    bi = pair
                    due = kc0 + sub - 1 if (w or sub or pair) else -1
                    sched[(bi, max(due, -1))].append(
                        vproj(w, sub, pair, 0, wv_n, bv_n))
                    sched[(bi, max(kc0 + sub, -1))].append(
                        vproj(w, sub, pair, 1, wv_n, bv_n))
        # q chains: qT[cc] block tc4 feeds block (tc4*2 + cc); each is
        # emitted in an earlier block (x-waves stay resident in SBUF)
        qdue = {(0, 0): ((0, -1), (0, -1)),
                (0, 1): ((0, 21), (0, 23)),
                (1, 0): ((1, 6), (1, 8)),
                (1, 1): ((1, 12), (1, 14)),
                (2, 0): ((2, 2), (2, 4)),
                (2, 1): ((3, 2), (3, 4)),
                (3, 0): ((4, 2), (4, 4)),
                (3, 1): ((5, 2), (5, 4))}
        for (tc4, cc), (due0, due1) in qdue.items():
            sched[due0].append(ckproj(tc4, cc, 0, qT_sb, 0, "wq", "bq"))
            sched[due1].append(ckproj(tc4, cc, 1, qT_sb, 0, "wq", "bq"))
        # weight DMAs: x-side + wq at the very start, ctx-side + wo a bit in
        sched[(0, -1)].insert(0, w_dma("wks", wks))
        sched[(0, -1)].insert(1, w_dma("wq", wq))
        sched[(0, -1)].insert(2, w_dma("wvs", wvs))
        sched[(0, 2)].insert(0, w_dma("wkc", wkc))
        sched[(0, 2)].insert(1, w_dma("wvc", wvc))
        sched[(0, 5)].insert(0, wo_dma)

        # ---- epilogue / out-projection closures ------------------------
        epilogue_state = {}

        def make_epilogue(pair, qs, pvA, pvB):
            def epiA():
                rt = misc_pool.tile([128, 1024], BF16, tag="recip")
                with nc.allow_low_precision(
                        reason="bf16 rounding of softmax reciprocal"):
                    nc.vector.reciprocal(rt[64:65, 0:512], pvA[64:65, :])
                pvs = misc_pool.tile([128, 512], F32, tag="pvsA")
                nc.vector.tensor_copy(pvs[0:64, :], pvA[0:64, :])
                bcA = ps_shared.tile([128, 512], F32, tag="ps")
                nc.tensor.matmul(
                    bcA[0:64, :], ones_sb[64:65, 0:64], rt[64:65, 0:512],
                    start=True, stop=True, tile_position=(64, 0))
                nc.vector.tensor_mul(
                    outT_sb[pair][0:64, qs], pvs[0:64, :], bcA[0:64, :])
                epilogue_state["rt"] = rt

            def epiB():
                rt = epilogue_state["rt"]
                with nc.allow_low_precision(
                        reason="bf16 rounding of softmax reciprocal"):
                    nc.vector.reciprocal(rt[64:65, 512:1024], pvB[64:65, :])
                pvs = misc_pool.tile([128, 512], F32, tag="pvsB")
                nc.vector.tensor_copy(pvs[0:64, :], pvB[0:64, :])
                bcB = ps_shared.tile([128, 512], F32, tag="ps")
                nc.tensor.matmul(
                    bcB[0:64, :], ones_sb[64:65, 0:64], rt[64:65, 512:1024],
                    start=True, stop=True, tile_position=(64, 0))
                odd = misc_pool.tile([128, 512], BF16, tag="odd")
                nc.vector.tensor_mul(
                    odd[0:64, :], pvs[0:64, :], bcB[0:64, :])
                # odd head lives at partitions 64-127 of outT: shift via DMA
                nc.sync.dma_start(out=outT_sb[pair][64:128, qs],
                                  in_=odd[0:64, :])

            return [epiA, epiB]

        def make_outproj(qc):
            work = []
            for qt in range(qc * 4, qc * 4 + 4):
                qsl = slice(qt * 128, (qt + 1) * 128)
                for fc in range(2):
                    fsl = slice(fc * 512, (fc + 1) * 512)

                    def mm(qsl=qsl, fsl=fsl):
                        ps = ps_shared.tile([128, 512], F32, tag="ps")
                        for cc in range(2):
                            nc.tensor.matmul(
                                ps, (outT_sb[cc][:, qsl]),
                                (wo_sb[:, cc, fsl]),
                                start=(cc == 0), stop=(cc == 1))
                        st = stage_pool.tile([128, 512], BF16, tag="stage")
                        nc.vector.tensor_copy(st, ps)
                        nc.sync.dma_start(out=out[qsl, fsl], in_=st)

                    work.append(mm)
            return work

        def emit_pv(pvA, pvB, hA, hB, pt, kc):
            # [V | ones] stationary: rows 0-63 = head out^T, row 64 =
            # softmax denominator
            vt = v_sb[kc]
            nc.tensor.matmul(
                pvA[0:65, :], vt[:, hA * 65:(hA + 1) * 65],
                pt[:, 0:512], start=(kc == 0), stop=(kc == 31))
            nc.tensor.matmul(
                pvB[0:65, :], vt[:, hB * 65:(hB + 1) * 65],
                pt[:, 512:1024], start=(kc == 0), stop=(kc == 31))

        # ---- attention spine -------------------------------------------
        for qc in range(4):
            qs = slice(qc * 512, (qc + 1) * 512)
            for pair in range(2):
                bi = qc * 2 + pair
                hA, hB = 2 * pair, 2 * pair + 1
                for f in sched.pop((bi, -1), ()):
                    f()
                pvA = ps_pv.tile([128, 512], F32, tag="pv")
                pvB = ps_pv.tile([128, 512], F32, tag="pv")
                prev_pt = None
                for kc in range(32):
                    ks = slice(kc * 128, (kc + 1) * 128)
                    s2 = ps_scores.tile([128, 1024], F32, tag="s")
                    nc.tensor.matmul(
                        s2[:, 0:512], (kT_sb[pair][0:64, ks]),
                        (qT_sb[pair][0:64, qs]), start=True, stop=True)
                    nc.tensor.matmul(
                        s2[:, 512:1024], (kT_sb[pair][64:128, ks]),
                        (qT_sb[pair][64:128, qs]), start=True, stop=True,
                        tile_position=(64, 0))
                    pt = p_pool.tile([128, 1024], BF16, tag="pt")
                    nc.scalar.activation(pt, s2, AFT.Exp)
                    # deadline work first; else drip one queued closure
                    due = sched.pop((bi, kc), None)
                    if due:
                        for f in due:
                            f()
                    elif pending:
                        pending.popleft()()
                    # PV lags one k-chunk so the PE never stalls on the exp
                    if prev_pt is not None:
                        emit_pv(pvA, pvB, hA, hB, prev_pt, kc - 1)
                    prev_pt = pt
                emit_pv(pvA, pvB, hA, hB, prev_pt, 31)
                epiA, epiB = make_epilogue(pair, qs, pvA, pvB)
                if bi < 7:
                    sched[(bi + 1, 0)].append(epiA)
                    sched[(bi + 1, 1)].append(epiB)
                else:
                    pending.extend([epiA, epiB])
            pending.extend(make_outproj(qc))
        assert not sched, f"unconsumed deadlines: {list(sched)}"
        while pending:
            pending.popleft()()


_NC_CACHE = None


def kernel(**inputs):
    global _NC_CACHE
    if _NC_CACHE is None:
        _NC_CACHE = build_nc()
    nc = _NC_CACHE

    f = {k: np.asarray(v, dtype=np.float32) for k, v in inputs.items()}
    x, context = f["x"], f["context"]
    B = x.shape[0]

    xTs = [np.ascontiguousarray(x[b].T).astype(BF) for b in range(B)]
    cTs = [np.ascontiguousarray(context[b].T).astype(BF) for b in range(B)]

    in_maps = []
    for b in range(B):
        for hg in range(4):
            sl = slice(hg * CS, (hg + 1) * CS)
            in_maps.append({
                "xT": xTs[b],
                "cT": cTs[b],
                "wq": (np.ascontiguousarray(f["Wq"][:, sl]) * 0.125).astype(BF),
                "wks": np.ascontiguousarray(f["Wks"][:, sl]).astype(BF),
                "wkc": np.ascontiguousarray(f["Wkc"][:, sl]).astype(BF),
                "wvs": np.ascontiguousarray(f["Wvs"][:, sl]).astype(BF),
                "wvc": np.ascontiguousarray(f["Wvc"][:, sl]).astype(BF),
                "bq": (f["bq"][sl] * 0.125).reshape(CS, 1).copy(),
                "bks": f["bks"][sl].reshape(CS, 1).copy(),
                "bkc": f["bkc"][sl].reshape(CS, 1).copy(),
                "bvs": f["bvs"][sl].reshape(1, CS).copy(),
                "bvc": f["bvc"][sl].reshape(1, CS).copy(),
                "wo": np.ascontiguousarray(f["Wo"][sl, :]).astype(BF),
            })

    res = run_bass_kernel_spmd(nc, in_maps, list(range(N_CORES))).results

    bo = f["bo"]
    out = np.empty((B, T, D), dtype=np.float32)
    for b in range(B):
        acc = res[b * 4 + 0]["out"].astype(np.float32)
        for hg in range(1, 4):
            acc += res[b * 4 + hg]["out"].astype(np.float32)
        out[b] = acc + bo
    return out


# revision 10
# speedup vs baseline: 1.0997x; 1.0063x over previous
"""JointAttention TRN2 Bass kernel.

Sharding: 8 cores = batch(2) x head-group(4). Each core owns one batch
element and 4 of the 16 heads (a 256-wide channel slice). All matmul
operands are bf16 (1 cyc/row on the PE at any free size); accumulation
stays fp32 in PSUM.

Per core:
  qT/kT projections in [c, t] layout (lhsT = W stationary, rhs = xT
  moving), v projection in [t, c] layout (lhsT = xT chunk stationary,
  rhs = W moving), scores^T = K^T.T @ Q^T per 128-key chunk ([k, q]
  layout, 2 heads row-tiled via tile_position), exp on ScalarE
  (activation engine is the critical resource: ~1.04us per [128,1024]
  tile), PV with V-augmented-ones columns giving the softmax
  denominators, division via a ones-matmul broadcast, and the output
  projection (row-parallel Wo slice).

Scheduling: everything except the QK->exp->PV spine is emitted through
a deadline-driven work queue that drips projection chains, softmax
epilogues and the output projection into the PE slack of the attention
k-iterations, so the activation engine starts exp-ing within a few us
of t=0 and never starves. The attention spine itself is software-
pipelined (PV lags QK by one k-chunk). The 4 partial outputs per batch
element are summed on the host (row-parallel all-reduce as part of
unsharding) and bo is added once.
"""

import sys
from collections import defaultdict, deque

import numpy as np

if "/opt/trn_rl_repo" not in sys.path:
    sys.path.insert(0, "/opt/trn_rl_repo")

import ml_dtypes

import concourse.bass as bass
import concourse.tile as tile
from concourse import bacc, mybir
from concourse.bass_utils import run_bass_kernel_spmd

F32 = mybir.dt.float32
BF16 = mybir.dt.bfloat16
AFT = mybir.ActivationFunctionType

D = 1024          # model dim
T = 2048          # query length (= self key length)
TK = 4096         # total key length (self + context)
CS = 256          # channels per core (4 heads x 64)
NH = 4            # heads per core
HD = 64           # head dim
DC = 8            # D chunks of 128
N_CORES = 8

BF = ml_dtypes.bfloat16


def build_nc():
    nc = bacc.Bacc(None)

    xT = nc.declare_dram_parameter("xT", [D, T], BF16, isOutput=False)
    cT = nc.declare_dram_parameter("cT", [D, T], BF16, isOutput=False)
    wq = nc.declare_dram_parameter("wq", [D, CS], BF16, isOutput=False)
    wks = nc.declare_dram_parameter("wks", [D, CS], BF16, isOutput=False)
    wkc = nc.declare_dram_parameter("wkc", [D, CS], BF16, isOutput=False)
    wvs = nc.declare_dram_parameter("wvs", [D, CS], BF16, isOutput=False)
    wvc = nc.declare_dram_parameter("wvc", [D, CS], BF16, isOutput=False)
    bq = nc.declare_dram_parameter("bq", [CS, 1], F32, isOutput=False)
    bks = nc.declare_dram_parameter("bks", [CS, 1], F32, isOutput=False)
    bkc = nc.declare_dram_parameter("bkc", [CS, 1], F32, isOutput=False)
    bvs = nc.declare_dram_parameter("bvs", [1, CS], F32, isOutput=False)
    bvc = nc.declare_dram_parameter("bvc", [1, CS], F32, isOutput=False)
    wo = nc.declare_dram_parameter("wo", [CS, D], BF16, isOutput=False)
    out = nc.declare_dram_parameter("out", [T, D], BF16, isOutput=True)

    with tile.TileContext(nc) as tc:
        _emit(nc, tc, xT, cT, wq, wks, wkc, wvs, wvc,
              bq, bks, bkc, bvs, bvc, wo, out)
    nc.compile()
    return nc


def _emit(nc, tc, xT, cT, wq, wks, wkc, wvs, wvc, bq, bks, bkc, bvs, bvc,
          wo, out):
    from contextlib import ExitStack

    ctx = ExitStack()
    with ctx:
        consts = ctx.enter_context(tc.tile_pool(name="consts", bufs=1))
        wpool = ctx.enter_context(tc.tile_pool(name="wpool", bufs=1))
        io_pool = ctx.enter_context(tc.tile_pool(name="io", bufs=64))
        qt_pool = ctx.enter_context(tc.tile_pool(name="qt", bufs=1))
        kt_pool = ctx.enter_context(tc.tile_pool(name="kt", bufs=1))
        v_pool = ctx.enter_context(tc.tile_pool(name="v", bufs=1))
        p_pool = ctx.enter_context(tc.tile_pool(name="p", bufs=4))
        outt_pool = ctx.enter_context(tc.tile_pool(name="outt", bufs=1))
        stage_pool = ctx.enter_context(tc.tile_pool(name="stage", bufs=3))
        misc_pool = ctx.enter_context(tc.tile_pool(name="misc", bufs=2))
        # PSUM: shared(2) + scores(4) + pv(2) = 8 banks
        ps_shared = ctx.enter_context(
            tc.tile_pool(name="ps_shared", bufs=2, space="PSUM"))
        ps_scores = ctx.enter_context(
            tc.tile_pool(name="ps_scores", bufs=2, space="PSUM"))
        ps_pv = ctx.enter_context(
            tc.tile_pool(name="ps_pv", bufs=2, space="PSUM"))

        # ---- small constants (cheap DMAs, emitted up front) ----
        b_sb = {}
        for name, b in (("bq", bq), ("bks", bks), ("bkc", bkc)):
            t = consts.tile([128, 2], F32, tag=f"b_{name}", name=f"b_{name}")
            nc.sync.dma_start(out=t, in_=b.rearrange("(a p) o -> p (a o)", p=128))
            b_sb[name] = t
        bv_sb = {}
        for name, b in (("bvs", bvs), ("bvc", bvc)):
            t = consts.tile([128, CS], F32, tag=f"bv_{name}", name=f"bv_{name}")
            nc.sync.dma_start(out=t, in_=b[:, :].to_broadcast([128, CS]))
            bv_sb[name] = t
        ones_sb = consts.tile([128, HD], BF16, tag="ones")
        nc.vector.memset(ones_sb[:], 1.0)

        # weight tiles: each gets its own slot (bf16 keeps SBUF cheap)
        w_sb = {}
        for name, w in (("wq", wq), ("wks", wks), ("wvs", wvs),
                        ("wkc", wkc), ("wvc", wvc)):
            w_sb[name] = wpool.tile([128, DC, CS], BF16, tag=f"w_{name}",
                                    name=f"w_{name}")
        wo_sb = consts.tile([128, 2, D], BF16, tag="wo")

        qT_sb = [qt_pool.tile([128, T], BF16, tag=f"qT{cc}", name=f"qT{cc}")
                 for cc in range(2)]
        kT_sb = [kt_pool.tile([128, TK], BF16, tag=f"kT{cc}", name=f"kT{cc}")
                 for cc in range(2)]
        v_sb = [v_pool.tile([128, NH * (HD + 1)], BF16, tag=f"v{kc}",
                            name=f"v{kc}")
                for kc in range(32)]
        outT_sb = [outt_pool.tile([128, T], BF16, tag=f"outT{cc}",
                                  name=f"outT{cc}")
                   for cc in range(2)]

        # ---- deadline-scheduled work ----------------------------------
        # sched[(bi, kc)] = closures that MUST be emitted right after the
        # exp of iteration (block bi, k-chunk kc); kc == -1 means at block
        # start, before its first QK.  `pending` holds order-only work
        # (epilogues, out-projection) popped one per iteration when no
        # deadline work is due.
        sched = defaultdict(list)
        pending = deque()

        srcs = ((xT, "wks", "wvs", "bks", "bvs"),
                (cT, "wkc", "wvc", "bkc", "bvc"))
        waves = [[None] * DC for _ in range(8)]  # 8 waves of 8 [128,512] tiles
        chain_ps = {}

        def io_dma(w):
            src = srcs[w // 4][0]
            tc4 = w % 4

            def go():
                for dc in range(DC):
                    t = io_pool.tile([128, 512], BF16, tag="io",
                                     name=f"io_{w}_{dc}")
                    nc.sync.dma_start(
                        out=t,
                        in_=src[dc * 128:(dc + 1) * 128,
                                tc4 * 512:(tc4 + 1) * 512])
                    waves[w][dc] = t

            go._tag = f"io_{w}"
            return go

        def w_dma(name, w):
            def go():
                nc.sync.dma_start(
                    out=w_sb[name], in_=w.rearrange("(a p) c -> p a c", p=128))

            return go

        def wo_dma():
            nc.sync.dma_start(
                out=wo_sb, in_=wo.rearrange("(a p) f -> p a f", p=128))

        def ckproj(w, cc, half, dst, coff, wn, bn):
            # half a [c,t]-projection chain (4 of 8 contraction steps);
            # halves share one PSUM accumulation group via chain_ps
            tc4 = w % 4

            def go():
                key = ("ck", w, cc, wn)
                if half == 0:
                    chain_ps[key] = ps_shared.tile(
                        [128, 512], F32, tag="ps", name=f"ps_ck_{w}_{cc}_{wn}")
                ps = chain_ps[key]
                for dc in range(half * 4, half * 4 + 4):
                    nc.tensor.matmul(
                        ps, (w_sb[wn][:, dc, cc * 128:(cc + 1) * 128]),
                        (waves[w][dc]),
                        start=(dc == 0), stop=(dc == DC - 1))
                if half == 1:
                    nc.vector.tensor_scalar_add(
                        dst[cc][:, coff + tc4 * 512:coff + (tc4 + 1) * 512],
                        ps, b_sb[bn][:, cc:cc + 1])

            go._tag = ("q0" if wn == "wq" else "k0") if (w == 0 and cc == 0) else ""
            go._half = half
            return go

        def vproj(w, sub, pair, half, wv_n, bv_n):
            # half a [t,c]-projection chain for one head pair (128 cols)
            kc = (w // 4) * 16 + (w % 4) * 4 + sub

            def go():
                key = ("v", w, sub, pair)
                if half == 0:
                    chain_ps[key] = ps_shared.tile(
                        [128, 512], F32, tag="ps", name=f"ps_v_{w}_{sub}_{pair}")
                ps = chain_ps[key]
                for dc in range(half * 4, half * 4 + 4):
                    nc.tensor.matmul(
                        ps[:, 0:128],
                        (waves[w][dc][:, sub * 128:(sub + 1) * 128]),
                        (w_sb[wv_n][:, dc, pair * 128:(pair + 1) * 128]),
                        start=(dc == 0), stop=(dc == DC - 1))
                if half == 1:
                    vt = v_sb[kc]
                    vt_v = vt[:].rearrange("p (h x) -> p h x", h=NH)
                    nc.vector.tensor_add(
                        vt_v[:, 2 * pair:2 * pair + 2, 0:HD],
                        ps[:, 0:128].rearrange("p (h x) -> p h x", h=2),
                        bv_sb[bv_n][:, pair * 128:(pair + 1) * 128]
                        .rearrange("p (h x) -> p h x", h=2))
                    nc.vector.memset(
                        vt_v[:, 2 * pair:2 * pair + 2, HD:HD + 1]
                        .rearrange("p h one -> p (h one)"), 1.0)

            go._tag = "v0" if (w == 0 and sub == 0 and pair == 0) else ""
            go._half = half
            return go

        # deadline assignment. blocks: bi = qc*2 + pair, 32 k-iterations
        # each.  An item due at (bi, kc) is emitted after exp(kc) and
        # before PV(kc-1) of that iteration.
        for w in range(8):
            src, wk_n, wv_n, bk_n, bv_n = srcs[w // 4]
            kc0 = (w // 4) * 16 + (w % 4) * 4  # first k-chunk of this wave
            # io prefetch ~6 iterations ahead of first use
            sched[(0, max(kc0 - 7, -1) if w else -1)].append(io_dma(w))
            # kT chains: cc=0 feeds block 0, cc=1 feeds block 1
            for cc in range(2):
                bi = cc
                due = max(kc0 - 2, -1) if (w or cc) else -1
                sched[(bi, due)].append(
                    ckproj(w, cc, 0, kT_sb, (w // 4) * T, wk_n, bk_n))
                sched[(bi, max(kc0 - 1, -1) if (w or cc) else -1)].append(
                    ckproj(w, cc, 1, kT_sb, (w // 4) * T, wk_n, bk_n))
            # v chains: pair p feeds block p, chunk kc0+sub needed by
            # PV at iteration kc0+sub+1
            for sub in range(4):
                for pair in range(2):
                    bi = pair
                    due = kc0 + sub - 1 if (w or sub or pair) else -1
                    sched[(bi, max(due, -1))].append(
                        vproj(w, sub, pair, 0, wv_n, bv_n))
                    sched[(bi, max(kc0 + sub, -1))].append(
                        vproj(w, sub, pair, 1, wv_n, bv_n))
        # q chains: qT[cc] block tc4 feeds block (tc4*2 + cc); each is
        # emitted in an earlier block (x-waves stay resident in SBUF)
        qdue = {(0, 0): ((0, -1), (0, -1)),
                (0, 1): ((0, 21), (0, 23)),
                (1, 0): ((1, 6), (1, 8)),
                (1, 1): ((1, 12), (1, 14)),
                (2, 0): ((2, 2), (2, 4)),
                (2, 1): ((3, 2), (3, 4)),
                (3, 0): ((4, 2), (4, 4)),
                (3, 1): ((5, 2), (5, 4))}
        for (tc4, cc), (due0, due1) in qdue.items():
            sched[due0].append(ckproj(tc4, cc, 0, qT_sb, 0, "wq", "bq"))
            sched[due1].append(ckproj(tc4, cc, 1, qT_sb, 0, "wq", "bq"))
        # weight DMAs: x-side + wq at the very start, ctx-side + wo a bit in
        wq_c, wks_c, wvs_c = w_dma("wq", wq), w_dma("wks", wks), w_dma("wvs", wvs)
        sched[(0, -1)].extend([wq_c, wks_c, wvs_c])
        sched[(0, 2)].insert(0, w_dma("wkc", wkc))
        sched[(0, 2)].insert(1, w_dma("wvc", wvc))
        sched[(0, 5)].insert(0, wo_dma)

        # prologue order: the chain gating the first exp goes first
        # (wq dma -> wave0 dma -> wks -> qproj -> kproj -> QK)
        first = {id(wq_c): 0, id(wks_c): 2, id(wvs_c): 7}
        for i, f in enumerate(sched[(0, -1)]):
            for nm, p in (("io_0", 1), ("q0", 3), ("k0", 5), ("v0", 8)):
                if getattr(f, "_tag", None) == nm:
                    first[id(f)] = p + getattr(f, "_half", 0)
        sched[(0, -1)].sort(key=lambda f: first.get(id(f), 50))

        # ---- epilogue / out-projection closures ------------------------
        epilogue_state = {}

        def make_epilogue(pair, qs, pvA, pvB):
            def epiA():
                rt = misc_pool.tile([128, 1024], BF16, tag="recip")
                with nc.allow_low_precision(
                        reason="bf16 rounding of softmax reciprocal"):
                    nc.vector.reciprocal(rt[64:65, 0:512], pvA[64:65, :])
                pvs = misc_pool.tile([128, 512], F32, tag="pvsA")
                nc.vector.tensor_copy(pvs[0:64, :], pvA[0:64, :])
                bcA = ps_shared.tile([128, 512], F32, tag="ps")
                nc.tensor.matmul(
                    bcA[0:64, :], ones_sb[64:65, 0:64], rt[64:65, 0:512],
                    start=True, stop=True, tile_position=(64, 0))
                nc.vector.tensor_mul(
                    outT_sb[pair][0:64, qs], pvs[0:64, :], bcA[0:64, :])
                epilogue_state["rt"] = rt

            def epiB():
                rt = epilogue_state["rt"]
                with nc.allow_low_precision(
                        reason="bf16 rounding of softmax reciprocal"):
                    nc.vector.reciprocal(rt[64:65, 512:1024], pvB[64:65, :])
                pvs = misc_pool.tile([128, 512], F32, tag="pvsB")
                nc.vector.tensor_copy(pvs[0:64, :], pvB[0:64, :])
                bcB = ps_shared.tile([128, 512], F32, tag="ps")
                nc.tensor.matmul(
                    bcB[0:64, :], ones_sb[64:65, 0:64], rt[64:65, 512:1024],
                    start=True, stop=True, tile_position=(64, 0))
                odd = misc_pool.tile([128, 512], BF16, tag="odd")
                nc.vector.tensor_mul(
                    odd[0:64, :], pvs[0:64, :], bcB[0:64, :])
                # odd head lives at partitions 64-127 of outT: shift via DMA
                nc.sync.dma_start(out=outT_sb[pair][64:128, qs],
                                  in_=odd[0:64, :])

            return [epiA, epiB]

        def make_outproj(qc):
            work = []
            for qt in range(qc * 4, qc * 4 + 4):
                qsl = slice(qt * 128, (qt + 1) * 128)
                for fc in range(2):
                    fsl = slice(fc * 512, (fc + 1) * 512)

                    def mm(qsl=qsl, fsl=fsl):
                        ps = ps_shared.tile([128, 512], F32, tag="ps")
                        for cc in range(2):
                            nc.tensor.matmul(
                                ps, (outT_sb[cc][:, qsl]),
                                (wo_sb[:, cc, fsl]),
                                start=(cc == 0), stop=(cc == 1))
                        st = stage_pool.tile([128, 512], BF16, tag="stage")
                        nc.vector.tensor_copy(st, ps)
                        nc.sync.dma_start(out=out[qsl, fsl], in_=st)

                    work.append(mm)
            return work

        def emit_pv(pvA, pvB, hA, hB, pt, kc):
            # [V | ones] stationary: rows 0-63 = head out^T, row 64 =
            # softmax denominator
            vt = v_sb[kc]
            nc.tensor.matmul(
                pvA[0:65, :], vt[:, hA * 65:(hA + 1) * 65],
                pt[:, 0:512], start=(kc == 0), stop=(kc == 31))
            nc.tensor.matmul(
                pvB[0:65, :], vt[:, hB * 65:(hB + 1) * 65],
                pt[:, 512:1024], start=(kc == 0), stop=(kc == 31))

        # ---- attention spine -------------------------------------------
        for qc in range(4):
            qs = slice(qc * 512, (qc + 1) * 512)
            for pair in range(2):
                bi = qc * 2 + pair
                hA, hB = 2 * pair, 2 * pair + 1
                for f in sched.pop((bi, -1), ()):
                    f()
                pvA = ps_pv.tile([128, 512], F32, tag="pv")
                pvB = ps_pv.tile([128, 512], F32, tag="pv")
                prev_pt = None
                for kc in range(32):
                    ks = slice(kc * 128, (kc + 1) * 128)
                    s2 = ps_scores.tile([128, 1024], F32, tag="s")
                    nc.tensor.matmul(
                        s2[:, 0:512], (kT_sb[pair][0:64, ks]),
                        (qT_sb[pair][0:64, qs]), start=True, stop=True)
                    nc.tensor.matmul(
                        s2[:, 512:1024], (kT_sb[pair][64:128, ks]),
                        (qT_sb[pair][64:128, qs]), start=True, stop=True,
                        tile_position=(64, 0))
                    pt = p_pool.tile([128, 1024], BF16, tag="pt")
                    nc.scalar.activation(pt, s2, AFT.Exp)
                    # deadline work first; else drip one queued closure
                    due = sched.pop((bi, kc), None)
                    if due:
                        for f in due:
                            f()
                    elif pending:
                        pending.popleft()()
                    # PV lags one k-chunk so the PE never stalls on the exp
                    if prev_pt is not None:
                        emit_pv(pvA, pvB, hA, hB, prev_pt, kc - 1)
                    prev_pt = pt
                emit_pv(pvA, pvB, hA, hB, prev_pt, 31)
                epiA, epiB = make_epilogue(pair, qs, pvA, pvB)
                if bi < 7:
                    sched[(bi + 1, 0)].append(epiA)
                    sched[(bi + 1, 1)].append(epiB)
                else:
                    pending.extend([epiA, epiB])
            pending.extend(make_outproj(qc))
        assert not sched, f"unconsumed deadlines: {list(sched)}"
        while pending:
            pending.popleft()()


_NC_CACHE = None


def kernel(**inputs):
    global _NC_CACHE
    if _NC_CACHE is None:
        _NC_CACHE = build_nc()
    nc = _NC_CACHE

    f = {k: np.asarray(v, dtype=np.float32) for k, v in inputs.items()}
    x, context = f["x"], f["context"]
    B = x.shape[0]

    xTs = [np.ascontiguousarray(x[b].T).astype(BF) for b in range(B)]
    cTs = [np.ascontiguousarray(context[b].T).astype(BF) for b in range(B)]

    in_maps = []
    for b in range(B):
        for hg in range(4):
            sl = slice(hg * CS, (hg + 1) * CS)
            in_maps.append({
                "xT": xTs[b],
                "cT": cTs[b],
                "wq": (np.ascontiguousarray(f["Wq"][:, sl]) * 0.125).astype(BF),
                "wks": np.ascontiguousarray(f["Wks"][:, sl]).astype(BF),
                "wkc": np.ascontiguousarray(f["Wkc"][:, sl]).astype(BF),
                "wvs": np.ascontiguousarray(f["Wvs"][:, sl]).astype(BF),
                "wvc": np.ascontiguousarray(f["Wvc"][:, sl]).astype(BF),
                "bq": (f["bq"][sl] * 0.125).reshape(CS, 1).copy(),
                "bks": f["bks"][sl].reshape(CS, 1).copy(),
                "bkc": f["bkc"][sl].reshape(CS, 1).copy(),
                "bvs": f["bvs"][sl].reshape(1, CS).copy(),
                "bvc": f["bvc"][sl].reshape(1, CS).copy(),
                "wo": np.ascontiguousarray(f["Wo"][sl, :]).astype(BF),
            })

    res = run_bass_kernel_spmd(nc, in_maps, list(range(N_CORES))).results

    bo = f["bo"]
    out = np.empty((B, T, D), dtype=np.float32)
    for b in range(B):
        acc = res[b * 4 + 0]["out"].astype(np.float32)
        for hg in range(1, 4):
            acc += res[b * 4 + hg]["out"].astype(np.float32)
        out[b] = acc + bo
    return out


# revision 11
# speedup vs baseline: 1.1100x; 1.0093x over previous
"""JointAttention TRN2 Bass kernel.

Sharding: 8 cores = batch(2) x head-group(4). Each core owns one batch
element and 4 of the 16 heads (a 256-wide channel slice). All matmul
operands are bf16 (1 cyc/row on the PE at any free size); accumulation
stays fp32 in PSUM.

Per core:
  qT/kT projections in [c, t] layout (lhsT = W stationary, rhs = xT
  moving), v projection in [t, c] layout (lhsT = xT chunk stationary,
  rhs = W moving), scores^T = K^T.T @ Q^T per 128-key chunk ([k, q]
  layout, 2 heads row-tiled via tile_position), exp on ScalarE
  (activation engine is the critical resource: ~1.04us per [128,1024]
  tile), PV with V-augmented-ones columns giving the softmax
  denominators, division via a ones-matmul broadcast, and the output
  projection (row-parallel Wo slice).

Scheduling: everything except the QK->exp->PV spine is emitted through
a deadline-driven work queue that drips projection chains, softmax
epilogues and the output projection into the PE slack of the attention
k-iterations, so the activation engine starts exp-ing within a few us
of t=0 and never starves. The attention spine itself is software-
pipelined (PV lags QK by one k-chunk). The 4 partial outputs per batch
element are summed on the host (row-parallel all-reduce as part of
unsharding) and bo is added once.
"""

import sys
from collections import defaultdict, deque

import numpy as np

if "/opt/trn_rl_repo" not in sys.path:
    sys.path.insert(0, "/opt/trn_rl_repo")

import ml_dtypes

import concourse.bass as bass
import concourse.tile as tile
from concourse import bacc, mybir
from concourse.bass_utils import run_bass_kernel_spmd

F32 = mybir.dt.float32
BF16 = mybir.dt.bfloat16
AFT = mybir.ActivationFunctionType

D = 1024          # model dim
T = 2048          # query length (= self key length)
TK = 4096         # total key length (self + context)
CS = 256          # channels per core (4 heads x 64)
NH = 4            # heads per core
HD = 64           # head dim
DC = 8            # D chunks of 128
N_CORES = 8

BF = ml_dtypes.bfloat16


def build_nc():
    nc = bacc.Bacc(None)

    xT = nc.declare_dram_parameter("xT", [D, T], BF16, isOutput=False)
    cT = nc.declare_dram_parameter("cT", [D, T], BF16, isOutput=False)
    wq = nc.declare_dram_parameter("wq", [D, CS], BF16, isOutput=False)
    wks = nc.declare_dram_parameter("wks", [D, CS], BF16, isOutput=False)
    wkc = nc.declare_dram_parameter("wkc", [D, CS], BF16, isOutput=False)
    wvs = nc.declare_dram_parameter("wvs", [D, CS], BF16, isOutput=False)
    wvc = nc.declare_dram_parameter("wvc", [D, CS], BF16, isOutput=False)
    bq = nc.declare_dram_parameter("bq", [CS, 1], F32, isOutput=False)
    bks = nc.declare_dram_parameter("bks", [CS, 1], F32, isOutput=False)
    bkc = nc.declare_dram_parameter("bkc", [CS, 1], F32, isOutput=False)
    bvs = nc.declare_dram_parameter("bvs", [1, CS], F32, isOutput=False)
    bvc = nc.declare_dram_parameter("bvc", [1, CS], F32, isOutput=False)
    wo = nc.declare_dram_parameter("wo", [CS, D], BF16, isOutput=False)
    out = nc.declare_dram_parameter("out", [T, D], BF16, isOutput=True)

    with tile.TileContext(nc) as tc:
        _emit(nc, tc, xT, cT, wq, wks, wkc, wvs, wvc,
              bq, bks, bkc, bvs, bvc, wo, out)
    nc.compile()
    return nc


def _emit(nc, tc, xT, cT, wq, wks, wkc, wvs, wvc, bq, bks, bkc, bvs, bvc,
          wo, out):
    from contextlib import ExitStack

    ctx = ExitStack()
    with ctx:
        consts = ctx.enter_context(tc.tile_pool(name="consts", bufs=1))
        wpool = ctx.enter_context(tc.tile_pool(name="wpool", bufs=1))
        io_pool = ctx.enter_context(tc.tile_pool(name="io", bufs=64))
        qt_pool = ctx.enter_context(tc.tile_pool(name="qt", bufs=1))
        kt_pool = ctx.enter_context(tc.tile_pool(name="kt", bufs=1))
        v_pool = ctx.enter_context(tc.tile_pool(name="v", bufs=1))
        p_pool = ctx.enter_context(tc.tile_pool(name="p", bufs=4))
        outt_pool = ctx.enter_context(tc.tile_pool(name="outt", bufs=1))
        stage_pool = ctx.enter_context(tc.tile_pool(name="stage", bufs=3))
        misc_pool = ctx.enter_context(tc.tile_pool(name="misc", bufs=2))
        # PSUM: shared(2) + scores(4) + pv(2) = 8 banks
        ps_shared = ctx.enter_context(
            tc.tile_pool(name="ps_shared", bufs=2, space="PSUM"))
        ps_scores = ctx.enter_context(
            tc.tile_pool(name="ps_scores", bufs=2, space="PSUM"))
        ps_pv = ctx.enter_context(
            tc.tile_pool(name="ps_pv", bufs=2, space="PSUM"))

        # ---- small constants (cheap DMAs, emitted up front) ----
        b_sb = {}
        for name, b in (("bq", bq), ("bks", bks), ("bkc", bkc)):
            t = consts.tile([128, 2], F32, tag=f"b_{name}", name=f"b_{name}")
            nc.sync.dma_start(out=t, in_=b.rearrange("(a p) o -> p (a o)", p=128))
            b_sb[name] = t
        bv_sb = {}
        for name, b in (("bvs", bvs), ("bvc", bvc)):
            t = consts.tile([128, CS], F32, tag=f"bv_{name}", name=f"bv_{name}")
            nc.sync.dma_start(out=t, in_=b[:, :].to_broadcast([128, CS]))
            bv_sb[name] = t
        ones_sb = consts.tile([128, HD], BF16, tag="ones")
        nc.vector.memset(ones_sb[:], 1.0)

        # weight tiles: each gets its own slot (bf16 keeps SBUF cheap)
        w_sb = {}
        for name, w in (("wq", wq), ("wks", wks), ("wvs", wvs),
                        ("wkc", wkc), ("wvc", wvc)):
            w_sb[name] = wpool.tile([128, DC, CS], BF16, tag=f"w_{name}",
                                    name=f"w_{name}")
        wo_sb = consts.tile([128, 2, D], BF16, tag="wo")

        qT_sb = [qt_pool.tile([128, T], BF16, tag=f"qT{cc}", name=f"qT{cc}")
                 for cc in range(2)]
        kT_sb = [kt_pool.tile([128, TK], BF16, tag=f"kT{cc}", name=f"kT{cc}")
                 for cc in range(2)]
        v_sb = [v_pool.tile([128, NH * (HD + 1)], BF16, tag=f"v{kc}",
                            name=f"v{kc}")
                for kc in range(32)]
        outT_sb = [outt_pool.tile([128, T], BF16, tag=f"outT{cc}",
                                  name=f"outT{cc}")
                   for cc in range(2)]

        # ---- deadline-scheduled work ----------------------------------
        # sched[(bi, kc)] = closures that MUST be emitted right after the
        # exp of iteration (block bi, k-chunk kc); kc == -1 means at block
        # start, before its first QK.  `pending` holds order-only work
        # (epilogues, out-projection) popped one per iteration when no
        # deadline work is due.
        sched = defaultdict(list)
        pending = deque()

        srcs = ((xT, "wks", "wvs", "bks", "bvs"),
                (cT, "wkc", "wvc", "bkc", "bvc"))
        waves = [[None] * DC for _ in range(8)]  # 8 waves of 8 [128,512] tiles
        chain_ps = {}

        def io_dma(w):
            src = srcs[w // 4][0]
            tc4 = w % 4

            def go():
                for dc in range(DC):
                    t = io_pool.tile([128, 512], BF16, tag="io",
                                     name=f"io_{w}_{dc}")
                    nc.sync.dma_start(
                        out=t,
                        in_=src[dc * 128:(dc + 1) * 128,
                                tc4 * 512:(tc4 + 1) * 512])
                    waves[w][dc] = t

            go._tag = f"io_{w}"
            return go

        def w_dma(name, w):
            def go():
                nc.sync.dma_start(
                    out=w_sb[name], in_=w.rearrange("(a p) c -> p a c", p=128))

            return go

        def wo_dma():
            nc.sync.dma_start(
                out=wo_sb, in_=wo.rearrange("(a p) f -> p a f", p=128))

        def ckproj(w, cc, half, dst, coff, wn, bn):
            # half a [c,t]-projection chain (4 of 8 contraction steps);
            # halves share one PSUM accumulation group via chain_ps
            tc4 = w % 4

            def go():
                key = ("ck", w, cc, wn)
                if half == 0:
                    chain_ps[key] = ps_shared.tile(
                        [128, 512], F32, tag="ps", name=f"ps_ck_{w}_{cc}_{wn}")
                ps = chain_ps[key]
                for dc in range(half * 4, half * 4 + 4):
                    nc.tensor.matmul(
                        ps, (w_sb[wn][:, dc, cc * 128:(cc + 1) * 128]),
                        (waves[w][dc]),
                        start=(dc == 0), stop=(dc == DC - 1))
                if half == 1:
                    nc.vector.tensor_scalar_add(
                        dst[cc][:, coff + tc4 * 512:coff + (tc4 + 1) * 512],
                        ps, b_sb[bn][:, cc:cc + 1])

            go._tag = ("q0" if wn == "wq" else "k0") if (w == 0 and cc == 0) else ""
            go._half = half
            return go

        def vproj(w, sub, pair, half, wv_n, bv_n):
            # half a [t,c]-projection chain for one head pair (128 cols)
            kc = (w // 4) * 16 + (w % 4) * 4 + sub

            def go():
                key = ("v", w, sub, pair)
                if half == 0:
                    chain_ps[key] = ps_shared.tile(
                        [128, 512], F32, tag="ps", name=f"ps_v_{w}_{sub}_{pair}")
                ps = chain_ps[key]
                for dc in range(half * 4, half * 4 + 4):
                    nc.tensor.matmul(
                        ps[:, 0:128],
                        (waves[w][dc][:, sub * 128:(sub + 1) * 128]),
                        (w_sb[wv_n][:, dc, pair * 128:(pair + 1) * 128]),
                        start=(dc == 0), stop=(dc == DC - 1))
                if half == 1:
                    vt = v_sb[kc]
                    vt_v = vt[:].rearrange("p (h x) -> p h x", h=NH)
                    nc.vector.tensor_add(
                        vt_v[:, 2 * pair:2 * pair + 2, 0:HD],
                        ps[:, 0:128].rearrange("p (h x) -> p h x", h=2),
                        bv_sb[bv_n][:, pair * 128:(pair + 1) * 128]
                        .rearrange("p (h x) -> p h x", h=2))
                    nc.vector.memset(
                        vt_v[:, 2 * pair:2 * pair + 2, HD:HD + 1]
                        .rearrange("p h one -> p (h one)"), 1.0)

            go._tag = "v0" if (w == 0 and sub == 0 and pair == 0) else ""
            go._half = half
            return go

        # deadline assignment. blocks: bi = qc*2 + pair, 32 k-iterations
        # each.  An item due at (bi, kc) is emitted after exp(kc) and
        # before PV(kc-1) of that iteration.
        for w in range(8):
            src, wk_n, wv_n, bk_n, bv_n = srcs[w // 4]
            kc0 = (w // 4) * 16 + (w % 4) * 4  # first k-chunk of this wave
            # io prefetch ~6 iterations ahead of first use
            sched[(0, max(kc0 - 7, -1) if w else -1)].append(io_dma(w))
            # kT chains: cc=0 feeds block 0, cc=1 feeds block 1
            for cc in range(2):
                bi = cc
                due = max(kc0 - 2, -1) if (w or cc) else -1
                sched[(bi, due)].append(
                    ckproj(w, cc, 0, kT_sb, (w // 4) * T, wk_n, bk_n))
                sched[(bi, max(kc0 - 1, -1) if (w or cc) else -1)].append(
                    ckproj(w, cc, 1, kT_sb, (w // 4) * T, wk_n, bk_n))
            # v chains: pair p feeds block p, chunk kc0+sub needed by
            # PV at iteration kc0+sub+1
            for sub in range(4):
                for pair in range(2):
                    bi = pair
                    due = kc0 + sub - 1 if (w or sub or pair) else -1
                    sched[(bi, max(due, -1))].append(
                        vproj(w, sub, pair, 0, wv_n, bv_n))
                    sched[(bi, max(kc0 + sub, -1))].append(
                        vproj(w, sub, pair, 1, wv_n, bv_n))
        # q chains: qT[cc] block tc4 feeds block (tc4*2 + cc); each is
        # emitted in an earlier block (x-waves stay resident in SBUF)
        qdue = {(0, 0): ((0, -1), (0, -1)),
                (0, 1): ((0, 21), (0, 23)),
                (1, 0): ((1, 6), (1, 8)),
                (1, 1): ((1, 12), (1, 14)),
                (2, 0): ((2, 2), (2, 4)),
                (2, 1): ((3, 2), (3, 4)),
                (3, 0): ((4, 2), (4, 4)),
                (3, 1): ((5, 2), (5, 4))}
        for (tc4, cc), (due0, due1) in qdue.items():
            sched[due0].append(ckproj(tc4, cc, 0, qT_sb, 0, "wq", "bq"))
            sched[due1].append(ckproj(tc4, cc, 1, qT_sb, 0, "wq", "bq"))
        # weight DMAs: x-side + wq at the very start, ctx-side + wo a bit in
        wq_c, wks_c, wvs_c = w_dma("wq", wq), w_dma("wks", wks), w_dma("wvs", wvs)
        sched[(0, -1)].extend([wq_c, wks_c, wvs_c])
        sched[(0, 2)].insert(0, w_dma("wkc", wkc))
        sched[(0, 2)].insert(1, w_dma("wvc", wvc))
        sched[(0, 5)].insert(0, wo_dma)

        # prologue order: the chain gating the first exp goes first
        # (wq dma -> wave0 dma -> wks -> qproj -> kproj -> QK)
        first = {id(wq_c): 0, id(wks_c): 2, id(wvs_c): 7}
        for i, f in enumerate(sched[(0, -1)]):
            for nm, p in (("io_0", 1), ("q0", 3), ("k0", 5), ("v0", 8)):
                if getattr(f, "_tag", None) == nm:
                    first[id(f)] = p + getattr(f, "_half", 0)
        sched[(0, -1)].sort(key=lambda f: first.get(id(f), 50))

        # ---- epilogue / out-projection closures ------------------------
        epilogue_state = {}

        def make_epilogue(pair, qs, pvA, pvB):
            def epiA():
                rt = misc_pool.tile([128, 1024], BF16, tag="recip")
                with nc.allow_low_precision(
                        reason="bf16 rounding of softmax reciprocal"):
                    nc.vector.reciprocal(rt[64:65, 0:512], pvA[64:65, :])
                pvs = misc_pool.tile([128, 512], F32, tag="pvsA")
                nc.vector.tensor_copy(pvs[0:64, :], pvA[0:64, :])
                bcA = ps_shared.tile([128, 512], F32, tag="ps")
                nc.tensor.matmul(
                    bcA[0:64, :], ones_sb[64:65, 0:64], rt[64:65, 0:512],
                    start=True, stop=True, tile_position=(64, 0))
                nc.vector.tensor_mul(
                    outT_sb[pair][0:64, qs], pvs[0:64, :], bcA[0:64, :])
                epilogue_state["rt"] = rt

            def epiB():
                rt = epilogue_state["rt"]
                with nc.allow_low_precision(
                        reason="bf16 rounding of softmax reciprocal"):
                    nc.vector.reciprocal(rt[64:65, 512:1024], pvB[64:65, :])
                pvs = misc_pool.tile([128, 512], F32, tag="pvsB")
                nc.vector.tensor_copy(pvs[0:64, :], pvB[0:64, :])
                bcB = ps_shared.tile([128, 512], F32, tag="ps")
                nc.tensor.matmul(
                    bcB[0:64, :], ones_sb[64:65, 0:64], rt[64:65, 512:1024],
                    start=True, stop=True, tile_position=(64, 0))
                odd = misc_pool.tile([128, 512], BF16, tag="odd")
                nc.vector.tensor_mul(
                    odd[0:64, :], pvs[0:64, :], bcB[0:64, :])
                # odd head lives at partitions 64-127 of outT: shift via DMA
                nc.sync.dma_start(out=outT_sb[pair][64:128, qs],
                                  in_=odd[0:64, :])

            return [epiA, epiB]

        def make_outproj(qc):
            work = []
            for qt in range(qc * 4, qc * 4 + 4):
                qsl = slice(qt * 128, (qt + 1) * 128)
                for fc in range(2):
                    fsl = slice(fc * 512, (fc + 1) * 512)

                    def mm(qsl=qsl, fsl=fsl):
                        ps = ps_shared.tile([128, 512], F32, tag="ps")
                        for cc in range(2):
                            nc.tensor.matmul(
                                ps, (outT_sb[cc][:, qsl]),
                                (wo_sb[:, cc, fsl]),
                                start=(cc == 0), stop=(cc == 1))
                        st = stage_pool.tile([128, 512], BF16, tag="stage")
                        nc.vector.tensor_copy(st, ps)
                        nc.sync.dma_start(out=out[qsl, fsl], in_=st)

                    work.append(mm)
            return work

        def emit_pv(pvA, pvB, hA, hB, pt, kc):
            # [V | ones] stationary: rows 0-63 = head out^T, row 64 =
            # softmax denominator
            vt = v_sb[kc]
            nc.tensor.matmul(
                pvA[0:65, :], vt[:, hA * 65:(hA + 1) * 65],
                pt[:, 0:512], start=(kc == 0), stop=(kc == 31))
            nc.tensor.matmul(
                pvB[0:65, :], vt[:, hB * 65:(hB + 1) * 65],
                pt[:, 512:1024], start=(kc == 0), stop=(kc == 31))

        # ---- attention spine -------------------------------------------
        # flat 256-iteration pipeline over (qc, pair, kc).  Iteration g:
        #   exp(g) -> deadline pops -> QK(g+1) -> PV(g-1)
        # QK runs a full iteration ahead of its exp, so the activation
        # engine never waits on PE pops; PV lags one iteration so it never
        # waits on the exp.  Deadline keys are global iteration indices;
        # block-start producers sit at the previous block's last iteration
        # (popped before that iteration emits the next block's first QK).
        iters = [(qc, pair, kc)
                 for qc in range(4) for pair in range(2) for kc in range(32)]
        gsched = {}
        for (bi, kc), fs in sched.items():
            g = bi * 32 + kc if kc >= 0 else bi * 32 - 1
            gsched.setdefault(g, []).extend(fs)
        sched.clear()

        s2_of = {}

        def emit_qk(g):
            qc, pair, kc = iters[g]
            qs = slice(qc * 512, (qc + 1) * 512)
            ks = slice(kc * 128, (kc + 1) * 128)
            s2 = ps_scores.tile([128, 1024], F32, tag="s", name=f"s2_{g}")
            nc.tensor.matmul(
                s2[:, 0:512], (kT_sb[pair][0:64, ks]),
                (qT_sb[pair][0:64, qs]), start=True, stop=True)
            nc.tensor.matmul(
                s2[:, 512:1024], (kT_sb[pair][64:128, ks]),
                (qT_sb[pair][64:128, qs]), start=True, stop=True,
                tile_position=(64, 0))
            s2_of[g] = s2

        def emit_pv(g, pv):
            # [V | ones] stationary: rows 0-63 = head out^T, row 64 =
            # softmax denominator
            pt, pvA, pvB, hA, hB, kc = pv
            vt = v_sb[kc]
            nc.tensor.matmul(
                pvA[0:65, :], vt[:, hA * 65:(hA + 1) * 65],
                pt[:, 0:512], start=(kc == 0), stop=(kc == 31))
            nc.tensor.matmul(
                pvB[0:65, :], vt[:, hB * 65:(hB + 1) * 65],
                pt[:, 512:1024], start=(kc == 0), stop=(kc == 31))

        for f in gsched.pop(-1, ()):
            f()
        emit_qk(0)
        prev = None
        blk = {}
        for g, (qc, pair, kc) in enumerate(iters):
            bi = qc * 2 + pair
            if kc == 0:
                pvA = ps_pv.tile([128, 512], F32, tag="pv", name=f"pvA{bi}")
                pvB = ps_pv.tile([128, 512], F32, tag="pv", name=f"pvB{bi}")
                blk = {"pvA": pvA, "pvB": pvB,
                       "hA": 2 * pair, "hB": 2 * pair + 1,
                       "qs": slice(qc * 512, (qc + 1) * 512)}
            pt = p_pool.tile([128, 1024], BF16, tag="pt", name=f"pt{g}")
            nc.scalar.activation(pt, s2_of.pop(g), AFT.Exp)
            due = gsched.pop(g, None)
            if due:
                for f in due:
                    f()
            elif pending:
                pending.popleft()()
            if g + 1 < len(iters):
                emit_qk(g + 1)
            if prev is not None:
                emit_pv(g, prev)
            prev = (pt, blk["pvA"], blk["pvB"], blk["hA"], blk["hB"], kc)
            if kc == 31:
                emit_pv(g, prev)
                prev = None
                epiA, epiB = make_epilogue(
                    pair, blk["qs"], blk["pvA"], blk["pvB"])
                if bi < 7:
                    gsched.setdefault((bi + 1) * 32, []).append(epiA)
                    gsched.setdefault((bi + 1) * 32 + 1, []).append(epiB)
                else:
                    pending.extend([epiA, epiB])
                if pair == 1:
                    pending.extend(make_outproj(qc))
        assert not gsched, f"unconsumed deadlines: {sorted(gsched)}"
        while pending:
            pending.popleft()()


_NC_CACHE = None


def kernel(**inputs):
    global _NC_CACHE
    if _NC_CACHE is None:
        _NC_CACHE = build_nc()
    nc = _NC_CACHE

    f = {k: np.asarray(v, dtype=np.float32) for k, v in inputs.items()}
    x, context = f["x"], f["context"]
    B = x.shape[0]

    xTs = [np.ascontiguousarray(x[b].T).astype(BF) for b in range(B)]
    cTs = [np.ascontiguousarray(context[b].T).astype(BF) for b in range(B)]

    in_maps = []
    for b in range(B):
        for hg in range(4):
            sl = slice(hg * CS, (hg + 1) * CS)
            in_maps.append({
                "xT": xTs[b],
                "cT": cTs[b],
                "wq": (np.ascontiguousarray(f["Wq"][:, sl]) * 0.125).astype(BF),
                "wks": np.ascontiguousarray(f["Wks"][:, sl]).astype(BF),
                "wkc": np.ascontiguousarray(f["Wkc"][:, sl]).astype(BF),
                "wvs": np.ascontiguousarray(f["Wvs"][:, sl]).astype(BF),
                "wvc": np.ascontiguousarray(f["Wvc"][:, sl]).astype(BF),
                "bq": (f["bq"][sl] * 0.125).reshape(CS, 1).copy(),
                "bks": f["bks"][sl].reshape(CS, 1).copy(),
                "bkc": f["bkc"][sl].reshape(CS, 1).copy(),
                "bvs": f["bvs"][sl].reshape(1, CS).copy(),
                "bvc": f["bvc"][sl].reshape(1, CS).copy(),
                "wo": np.ascontiguousarray(f["Wo"][sl, :]).astype(BF),
            })

    res = run_bass_kernel_spmd(nc, in_maps, list(range(N_CORES))).results

    bo = f["bo"]
    out = np.empty((B, T, D), dtype=np.float32)
    for b in range(B):
        acc = res[b * 4 + 0]["out"].astype(np.float32)
        for hg in range(1, 4):
            acc += res[b * 4 + hg]["out"].astype(np.float32)
        out[b] = acc + bo
    return out


# revision 12
# speedup vs baseline: 1.2085x; 1.0887x over previous
"""JointAttention TRN2 Bass kernel.

Sharding: 8 cores = batch(2) x head-group(4). Each core owns one batch
element and 4 of the 16 heads (a 256-wide channel slice). All matmul
operands are bf16 (1 cyc/row on the PE at any free size); accumulation
stays fp32 in PSUM.

Per core:
  qT/kT projections in [c, t] layout (lhsT = W stationary, rhs = xT
  moving), v projection in [t, c] layout (lhsT = xT chunk stationary,
  rhs = W moving), scores^T = K^T.T @ Q^T per 128-key chunk ([k, q]
  layout, 2 heads row-tiled via tile_position), exp on ScalarE
  (activation engine is the critical resource: ~1.04us per [128,1024]
  tile), PV with V-augmented-ones columns giving the softmax
  denominators, division via a ones-matmul broadcast, and the output
  projection (row-parallel Wo slice).

Scheduling: everything except the QK->exp->PV spine is emitted through
a deadline-driven work queue that drips projection chains, softmax
epilogues and the output projection into the PE slack of the attention
k-iterations, so the activation engine starts exp-ing within a few us
of t=0 and never starves. The attention spine itself is software-
pipelined (PV lags QK by one k-chunk). The 4 partial outputs per batch
element are summed on the host (row-parallel all-reduce as part of
unsharding) and bo is added once.
"""

import sys
from collections import defaultdict, deque

import numpy as np

if "/opt/trn_rl_repo" not in sys.path:
    sys.path.insert(0, "/opt/trn_rl_repo")

import ml_dtypes

import concourse.bass as bass
import concourse.tile as tile
from concourse import bacc, mybir
from concourse.bass_utils import run_bass_kernel_spmd

F32 = mybir.dt.float32
BF16 = mybir.dt.bfloat16
AFT = mybir.ActivationFunctionType

D = 1024          # model dim
T = 2048          # query length (= self key length)
TK = 4096         # total key length (self + context)
CS = 256          # channels per core (4 heads x 64)
NH = 4            # heads per core
HD = 64           # head dim
DC = 8            # D chunks of 128
N_CORES = 8

BF = ml_dtypes.bfloat16


def build_nc():
    nc = bacc.Bacc(None)

    xT = nc.declare_dram_parameter("xT", [D, T], BF16, isOutput=False)
    cT = nc.declare_dram_parameter("cT", [D, T], BF16, isOutput=False)
    wq = nc.declare_dram_parameter("wq", [D, CS], BF16, isOutput=False)
    wks = nc.declare_dram_parameter("wks", [D, CS], BF16, isOutput=False)
    wkc = nc.declare_dram_parameter("wkc", [D, CS], BF16, isOutput=False)
    wvs = nc.declare_dram_parameter("wvs", [D, CS], BF16, isOutput=False)
    wvc = nc.declare_dram_parameter("wvc", [D, CS], BF16, isOutput=False)
    bq = nc.declare_dram_parameter("bq", [CS, 1], F32, isOutput=False)
    bks = nc.declare_dram_parameter("bks", [CS, 1], F32, isOutput=False)
    bkc = nc.declare_dram_parameter("bkc", [CS, 1], F32, isOutput=False)
    bvs = nc.declare_dram_parameter("bvs", [1, CS], F32, isOutput=False)
    bvc = nc.declare_dram_parameter("bvc", [1, CS], F32, isOutput=False)
    wo = nc.declare_dram_parameter("wo", [CS, D], BF16, isOutput=False)
    out = nc.declare_dram_parameter("out", [T, D], BF16, isOutput=True)

    with tile.TileContext(nc) as tc:
        _emit(nc, tc, xT, cT, wq, wks, wkc, wvs, wvc,
              bq, bks, bkc, bvs, bvc, wo, out)
    nc.compile()
    return nc


def _emit(nc, tc, xT, cT, wq, wks, wkc, wvs, wvc, bq, bks, bkc, bvs, bvc,
          wo, out):
    from contextlib import ExitStack

    ctx = ExitStack()
    with ctx:
        consts = ctx.enter_context(tc.tile_pool(name="consts", bufs=1))
        wpool = ctx.enter_context(tc.tile_pool(name="wpool", bufs=1))
        io_pool = ctx.enter_context(tc.tile_pool(name="io", bufs=64))
        qt_pool = ctx.enter_context(tc.tile_pool(name="qt", bufs=1))
        kt_pool = ctx.enter_context(tc.tile_pool(name="kt", bufs=1))
        v_pool = ctx.enter_context(tc.tile_pool(name="v", bufs=1))
        p_pool = ctx.enter_context(tc.tile_pool(name="p", bufs=16))
        outt_pool = ctx.enter_context(tc.tile_pool(name="outt", bufs=1))
        stage_pool = ctx.enter_context(tc.tile_pool(name="stage", bufs=3))
        misc_pool = ctx.enter_context(tc.tile_pool(name="misc", bufs=4))
        acc_pool = ctx.enter_context(tc.tile_pool(name="acc", bufs=2))
        an_pool = ctx.enter_context(tc.tile_pool(name="an", bufs=8))
        # PSUM: shared(2) + scores(4) + pv(2) = 8 banks
        ps_shared = ctx.enter_context(
            tc.tile_pool(name="ps_shared", bufs=2, space="PSUM"))
        ps_scores = ctx.enter_context(
            tc.tile_pool(name="ps_scores", bufs=2, space="PSUM"))
        ps_pv = ctx.enter_context(
            tc.tile_pool(name="ps_pv", bufs=2, space="PSUM"))

        # ---- small constants (cheap DMAs, emitted up front) ----
        b_sb = {}
        for name, b in (("bq", bq), ("bks", bks), ("bkc", bkc)):
            t = consts.tile([128, 2], F32, tag=f"b_{name}", name=f"b_{name}")
            nc.sync.dma_start(out=t, in_=b.rearrange("(a p) o -> p (a o)", p=128))
            b_sb[name] = t
        bv_sb = {}
        for name, b in (("bvs", bvs), ("bvc", bvc)):
            t = consts.tile([128, CS], F32, tag=f"bv_{name}", name=f"bv_{name}")
            nc.sync.dma_start(out=t, in_=b[:, :].to_broadcast([128, CS]))
            bv_sb[name] = t

        # weight tiles: each gets its own slot (bf16 keeps SBUF cheap)
        w_sb = {}
        for name, w in (("wq", wq), ("wks", wks), ("wvs", wvs),
                        ("wkc", wkc), ("wvc", wvc)):
            w_sb[name] = wpool.tile([128, DC, CS], BF16, tag=f"w_{name}",
                                    name=f"w_{name}")
        wo_sb = consts.tile([128, 2, D], BF16, tag="wo")

        qT_sb = [qt_pool.tile([128, T], BF16, tag=f"qT{cc}", name=f"qT{cc}")
                 for cc in range(2)]
        kT_sb = [kt_pool.tile([128, TK], BF16, tag=f"kT{cc}", name=f"kT{cc}")
                 for cc in range(2)]
        v_sb = [v_pool.tile([128, NH * (HD + 1)], BF16, tag=f"v{kc}",
                            name=f"v{kc}")
                for kc in range(32)]
        outT_sb = [outt_pool.tile([128, T], BF16, tag=f"outT{cc}",
                                  name=f"outT{cc}")
                   for cc in range(2)]

        # ---- deadline-scheduled work ----------------------------------
        # sched[(bi, kc)] = closures that MUST be emitted right after the
        # exp of iteration (block bi, k-chunk kc); kc == -1 means at block
        # start, before its first QK.  `pending` holds order-only work
        # (epilogues, out-projection) popped one per iteration when no
        # deadline work is due.
        sched = defaultdict(list)
        pending = deque()

        srcs = ((xT, "wks", "wvs", "bks", "bvs"),
                (cT, "wkc", "wvc", "bkc", "bvc"))
        waves = [[None] * DC for _ in range(8)]  # 8 waves of 8 [128,512] tiles
        chain_ps = {}

        def io_dma(w):
            src = srcs[w // 4][0]
            tc4 = w % 4

            def go():
                for dc in range(DC):
                    t = io_pool.tile([128, 512], BF16, tag="io",
                                     name=f"io_{w}_{dc}")
                    nc.sync.dma_start(
                        out=t,
                        in_=src[dc * 128:(dc + 1) * 128,
                                tc4 * 512:(tc4 + 1) * 512])
                    waves[w][dc] = t

            go._tag = f"io_{w}"
            return go

        def w_dma(name, w):
            def go():
                nc.sync.dma_start(
                    out=w_sb[name], in_=w.rearrange("(a p) c -> p a c", p=128))

            return go

        def wo_dma():
            nc.sync.dma_start(
                out=wo_sb, in_=wo.rearrange("(a p) f -> p a f", p=128))

        def ckproj(w, cc, half, dst, coff, wn, bn):
            # half a [c,t]-projection chain (4 of 8 contraction steps);
            # halves share one PSUM accumulation group via chain_ps
            tc4 = w % 4

            def go():
                key = ("ck", w, cc, wn)
                if half == 0:
                    chain_ps[key] = ps_shared.tile(
                        [128, 512], F32, tag="ps", name=f"ps_ck_{w}_{cc}_{wn}")
                ps = chain_ps[key]
                for dc in range(half * 4, half * 4 + 4):
                    nc.tensor.matmul(
                        ps, (w_sb[wn][:, dc, cc * 128:(cc + 1) * 128]),
                        (waves[w][dc]),
                        start=(dc == 0), stop=(dc == DC - 1))
                if half == 1:
                    nc.vector.tensor_scalar_add(
                        dst[cc][:, coff + tc4 * 512:coff + (tc4 + 1) * 512],
                        ps, b_sb[bn][:, cc:cc + 1])

            go._tag = ("q0" if wn == "wq" else "k0") if (w == 0 and cc == 0) else ""
            go._half = half
            return go

        def vproj(w, sub, pair, half, wv_n, bv_n):
            # half a [t,c]-projection chain for one head pair (128 cols)
            kc = (w // 4) * 16 + (w % 4) * 4 + sub

            def go():
                key = ("v", w, sub, pair)
                if half == 0:
                    chain_ps[key] = ps_shared.tile(
                        [128, 512], F32, tag="ps", name=f"ps_v_{w}_{sub}_{pair}")
                ps = chain_ps[key]
                for dc in range(half * 4, half * 4 + 4):
                    nc.tensor.matmul(
                        ps[:, 0:128],
                        (waves[w][dc][:, sub * 128:(sub + 1) * 128]),
                        (w_sb[wv_n][:, dc, pair * 128:(pair + 1) * 128]),
                        start=(dc == 0), stop=(dc == DC - 1))
                if half == 1:
                    vt = v_sb[kc]
                    vt_v = vt[:].rearrange("p (h x) -> p h x", h=NH)
                    nc.vector.tensor_add(
                        vt_v[:, 2 * pair:2 * pair + 2, 0:HD],
                        ps[:, 0:128].rearrange("p (h x) -> p h x", h=2),
                        bv_sb[bv_n][:, pair * 128:(pair + 1) * 128]
                        .rearrange("p (h x) -> p h x", h=2))
                    nc.vector.memset(
                        vt_v[:, 2 * pair:2 * pair + 2, HD:HD + 1]
                        .rearrange("p h one -> p (h one)"), 1.0)

            go._tag = "v0" if (w == 0 and sub == 0 and pair == 0) else ""
            go._half = half
            return go

        # deadline assignment. blocks: bi = qc*2 + pair, 32 k-iterations
        # each.  An item due at (bi, kc) is emitted after exp(kc) and
        # before PV(kc-1) of that iteration.
        for w in range(8):
            src, wk_n, wv_n, bk_n, bv_n = srcs[w // 4]
            kc0 = (w // 4) * 16 + (w % 4) * 4  # first k-chunk of this wave
            # io prefetch ~6 iterations ahead of first use
            sched[(0, max(kc0 - 7, -1) if w else -1)].append(io_dma(w))
            # kT chains: cc=0 feeds block 0, cc=1 feeds block 1
            for cc in range(2):
                bi = cc
                due = max(kc0 - 2, -1) if (w or cc) else -1
                sched[(bi, due)].append(
                    ckproj(w, cc, 0, kT_sb, (w // 4) * T, wk_n, bk_n))
                sched[(bi, max(kc0 - 1, -1) if (w or cc) else -1)].append(
                    ckproj(w, cc, 1, kT_sb, (w // 4) * T, wk_n, bk_n))
            # v chains: pair p feeds block p, chunk kc0+sub needed by
            # PV at iteration kc0+sub+1
            for sub in range(4):
                for pair in range(2):
                    bi = pair
                    due = kc0 + sub - 1 if (w or sub or pair) else -1
                    sched[(bi, max(due, -1))].append(
                        vproj(w, sub, pair, 0, wv_n, bv_n))
                    sched[(bi, max(kc0 + sub, -1))].append(
                        vproj(w, sub, pair, 1, wv_n, bv_n))
        # q chains: qT[cc] block tc4 feeds block (tc4*2 + cc); each is
        # emitted in an earlier block (x-waves stay resident in SBUF)
        qdue = {(0, 0): ((0, -1), (0, -1)),
                (0, 1): ((0, 21), (0, 23)),
                (1, 0): ((1, 6), (1, 8)),
                (1, 1): ((1, 12), (1, 14)),
                (2, 0): ((2, 2), (2, 4)),
                (2, 1): ((3, 2), (3, 4)),
                (3, 0): ((4, 2), (4, 4)),
                (3, 1): ((5, 2), (5, 4))}
        for (tc4, cc), (due0, due1) in qdue.items():
            sched[due0].append(ckproj(tc4, cc, 0, qT_sb, 0, "wq", "bq"))
            sched[due1].append(ckproj(tc4, cc, 1, qT_sb, 0, "wq", "bq"))
        # weight DMAs: x-side + wq at the very start, ctx-side + wo a bit in
        wq_c, wks_c, wvs_c = w_dma("wq", wq), w_dma("wks", wks), w_dma("wvs", wvs)
        sched[(0, -1)].extend([wq_c, wks_c, wvs_c])
        sched[(0, 2)].insert(0, w_dma("wkc", wkc))
        sched[(0, 2)].insert(1, w_dma("wvc", wvc))
        sched[(0, 5)].insert(0, wo_dma)

        # prologue order: the chain gating the first exp goes first
        # (wq dma -> wave0 dma -> wks -> qproj -> kproj -> QK)
        first = {id(wq_c): 0, id(wks_c): 2, id(wvs_c): 7}
        for i, f in enumerate(sched[(0, -1)]):
            for nm, p in (("io_0", 1), ("q0", 3), ("k0", 5), ("v0", 8)):
                if getattr(f, "_tag", None) == nm:
                    first[id(f)] = p + getattr(f, "_half", 0)
        sched[(0, -1)].sort(key=lambda f: first.get(id(f), 50))

        # ---- out-projection closures -----------------------------------
        def make_outproj(qc):
            work = []
            for qt in range(qc * 4, qc * 4 + 4):
                qsl = slice(qt * 128, (qt + 1) * 128)
                for fc in range(2):
                    fsl = slice(fc * 512, (fc + 1) * 512)

                    def mm(qsl=qsl, fsl=fsl):
                        ps = ps_shared.tile([128, 512], F32, tag="ps",
                                            name="ps_op")
                        for cc in range(2):
                            nc.tensor.matmul(
                                ps, (outT_sb[cc][:, qsl]),
                                (wo_sb[:, cc, fsl]),
                                start=(cc == 0), stop=(cc == 1))
                        st = stage_pool.tile([128, 512], BF16, tag="stage",
                                             name="st_op")
                        nc.vector.tensor_copy(st, ps)
                        nc.sync.dma_start(out=out[qsl, fsl], in_=st)

                    work.append(mm)
            return work

        # ---- attention spine -------------------------------------------
        # flat 256-iteration pipeline over (qc, pair, kc).  Iteration g:
        #   exp(g) -> deadline pops -> QK(g+1)
        # QK runs a full iteration ahead of its exp so the activation
        # engine never waits on PE work.  PV is restructured: the exp
        # tiles (pt, a 16-deep ring) become the matmul STATIONARY operand
        # and the [V|ones] columns the 65-row moving operand, producing
        # [128q, 65] PSUM tiles -- 65 PE rows per (head, q-block, k-chunk)
        # instead of 512/2.  Accumulation runs in 8-k-chunk segments
        # (2 rotating PSUM banks, one open group at a time) flushed into an
        # SBUF fp32 accumulator; the softmax division is then a
        # per-partition reciprocal+scale on the DVE, and the [q, c]->[c, q]
        # transpose into outT is done by the DMA crossbar
        # (dma_start_transpose), costing no engine time at all.
        iters = [(qc, pair, kc)
                 for qc in range(4) for pair in range(2) for kc in range(32)]
        gsched = {}
        for (bi, kc), fs in sched.items():
            g = bi * 32 + kc if kc >= 0 else bi * 32 - 1
            gsched.setdefault(g, []).extend(fs)
        sched.clear()

        s2_of = {}
        pt_of = {}
        an_of = {}

        def emit_qk(g):
            qc, pair, kc = iters[g]
            qs = slice(qc * 512, (qc + 1) * 512)
            ks = slice(kc * 128, (kc + 1) * 128)
            s2 = ps_scores.tile([128, 1024], F32, tag="s", name=f"s2_{g}")
            nc.tensor.matmul(
                s2[:, 0:512], (kT_sb[pair][0:64, ks]),
                (qT_sb[pair][0:64, qs]), start=True, stop=True)
            nc.tensor.matmul(
                s2[:, 512:1024], (kT_sb[pair][64:128, ks]),
                (qT_sb[pair][64:128, qs]), start=True, stop=True,
                tile_position=(64, 0))
            s2_of[g] = s2

        def make_pv_seg(bi, pair, seg, h, qb, acc):
            def go():
                pv = ps_pv.tile([128, 512], F32, tag="pv",
                                name=f"pv_{bi}_{seg}_{h}_{qb}")
                for j in range(8):
                    kc = seg * 8 + j
                    pt = pt_of[bi * 32 + kc]
                    nc.tensor.matmul(
                        pv[:, 0:65],
                        (pt[:, h * 512 + qb * 128:h * 512 + (qb + 1) * 128]),
                        (v_sb[kc][:, (2 * pair + h) * 65:
                                  (2 * pair + h + 1) * 65]),
                        start=(j == 0), stop=(j == 7))
                dst = acc[:, (h * 4 + qb) * 65:(h * 4 + qb + 1) * 65]
                if seg == 0:
                    nc.vector.tensor_copy(dst, pv[:, 0:65])
                else:
                    nc.vector.tensor_add(dst, dst, pv[:, 0:65])

            return go

        def make_div(bi, qc, pair, h, qb, acc):
            def go():
                base = (h * 4 + qb) * 65
                if h == 0:
                    an_of[(bi, qb)] = an_pool.tile(
                        [128, 128], BF16, tag="an", name=f"an_{bi}_{qb}")
                an = an_of[(bi, qb)]
                r = misc_pool.tile([128, 1], F32, tag="r",
                                   name=f"r_{bi}_{h}_{qb}")
                nc.vector.reciprocal(r, acc[:, base + 64:base + 65])
                nc.vector.tensor_scalar_mul(
                    an[:, h * 64:(h + 1) * 64],
                    acc[:, base:base + 64], r)

            return go

        def make_tp(bi, qc, pair, qb):
            def go():
                nc.sync.dma_start_transpose(
                    out=outT_sb[pair][:, qc * 512 + qb * 128:
                                      qc * 512 + (qb + 1) * 128],
                    in_=an_of[(bi, qb)][:])

            return go

        for f in gsched.pop(-1, ()):
            f()
        emit_qk(0)
        acc = None
        for g, (qc, pair, kc) in enumerate(iters):
            bi = qc * 2 + pair
            if kc == 0:
                acc = acc_pool.tile([128, 8 * 65], F32, tag="acc",
                                    name=f"acc_{bi}")
            pt = p_pool.tile([128, 1024], BF16, tag="pt", name=f"pt{g}")
            nc.scalar.activation(pt, s2_of.pop(g), AFT.Exp)
            pt_of[g] = pt
            due = gsched.pop(g, None)
            if due:
                for f in due:
                    f()
            elif pending:
                pending.popleft()()
            if g + 1 < len(iters):
                emit_qk(g + 1)
            if kc % 8 == 7:
                # schedule this segment's 8 PV groups over the next 8 iters
                seg = kc // 8
                for i, (h, qb) in enumerate(
                        (h, qb) for h in range(2) for qb in range(4)):
                    gsched.setdefault(g + 1 + i, []).append(
                        make_pv_seg(bi, pair, seg, h, qb, acc))
            if kc == 31:
                # divisions after the last flush, transposes after those,
                # out-projection once both pairs of this qc are transposed
                for i, (h, qb) in enumerate(
                        (h, qb) for h in range(2) for qb in range(4)):
                    gsched.setdefault(g + 9 + i, []).append(
                        make_div(bi, qc, pair, h, qb, acc))
                for qb in range(4):
                    gsched.setdefault(g + 17 + qb, []).append(
                        make_tp(bi, qc, pair, qb))
                if pair == 1:
                    pending.extend(make_outproj(qc))
        # drain: deadlines past the last iteration, then order-only work
        for g in sorted(gsched):
            for f in gsched.pop(g):
                f()
        while pending:
            pending.popleft()()


_NC_CACHE = None


def kernel(**inputs):
    global _NC_CACHE
    if _NC_CACHE is None:
        _NC_CACHE = build_nc()
    nc = _NC_CACHE

    f = {k: np.asarray(v, dtype=np.float32) for k, v in inputs.items()}
    x, context = f["x"], f["context"]
    B = x.shape[0]

    xTs = [np.ascontiguousarray(x[b].T).astype(BF) for b in range(B)]
    cTs = [np.ascontiguousarray(context[b].T).astype(BF) for b in range(B)]

    in_maps = []
    for b in range(B):
        for hg in range(4):
            sl = slice(hg * CS, (hg + 1) * CS)
            in_maps.append({
                "xT": xTs[b],
                "cT": cTs[b],
                "wq": (np.ascontiguousarray(f["Wq"][:, sl]) * 0.125).astype(BF),
                "wks": np.ascontiguousarray(f["Wks"][:, sl]).astype(BF),
                "wkc": np.ascontiguousarray(f["Wkc"][:, sl]).astype(BF),
                "wvs": np.ascontiguousarray(f["Wvs"][:, sl]).astype(BF),
                "wvc": np.ascontiguousarray(f["Wvc"][:, sl]).astype(BF),
                "bq": (f["bq"][sl] * 0.125).reshape(CS, 1).copy(),
                "bks": f["bks"][sl].reshape(CS, 1).copy(),
                "bkc": f["bkc"][sl].reshape(CS, 1).copy(),
                "bvs": f["bvs"][sl].reshape(1, CS).copy(),
                "bvc": f["bvc"][sl].reshape(1, CS).copy(),
                "wo": np.ascontiguousarray(f["Wo"][sl, :]).astype(BF),
            })

    res = run_bass_kernel_spmd(nc, in_maps, list(range(N_CORES))).results

    bo = f["bo"]
    out = np.empty((B, T, D), dtype=np.float32)
    for b in range(B):
        acc = res[b * 4 + 0]["out"].astype(np.float32)
        for hg in range(1, 4):
            acc += res[b * 4 + hg]["out"].astype(np.float32)
        out[b] = acc + bo
    return out


# revision 14
# speedup vs baseline: 1.2175x; 1.0074x over previous
"""JointAttention TRN2 Bass kernel.

Sharding: 8 cores = batch(2) x head-group(4). Each core owns one batch
element and 4 of the 16 heads (a 256-wide channel slice). All matmul
operands are bf16 (1 cyc/row on the PE at any free size); accumulation
stays fp32 in PSUM.

Per core:
  qT/kT projections in [c, t] layout (lhsT = W stationary, rhs = xT
  moving), v projection in [t, c] layout (lhsT = xT chunk stationary,
  rhs = W moving), scores^T = K^T.T @ Q^T per 128-key chunk ([k, q]
  layout, 2 heads row-tiled via tile_position), exp on ScalarE
  (activation engine is the critical resource: ~1.04us per [128,1024]
  tile), PV with V-augmented-ones columns giving the softmax
  denominators, division via a ones-matmul broadcast, and the output
  projection (row-parallel Wo slice).

Scheduling: everything except the QK->exp->PV spine is emitted through
a deadline-driven work queue that drips projection chains, softmax
epilogues and the output projection into the PE slack of the attention
k-iterations, so the activation engine starts exp-ing within a few us
of t=0 and never starves. The attention spine itself is software-
pipelined (PV lags QK by one k-chunk). The 4 partial outputs per batch
element are summed on the host (row-parallel all-reduce as part of
unsharding) and bo is added once.
"""

import sys
from collections import defaultdict, deque

import numpy as np

if "/opt/trn_rl_repo" not in sys.path:
    sys.path.insert(0, "/opt/trn_rl_repo")

import ml_dtypes

import concourse.bass as bass
import concourse.tile as tile
from concourse import bacc, mybir
from concourse.bass_utils import run_bass_kernel_spmd

F32 = mybir.dt.float32
BF16 = mybir.dt.bfloat16
AFT = mybir.ActivationFunctionType

D = 1024          # model dim
T = 2048          # query length (= self key length)
TK = 4096         # total key length (self + context)
CS = 256          # channels per core (4 heads x 64)
NH = 4            # heads per core
HD = 64           # head dim
DC = 8            # D chunks of 128
N_CORES = 8

BF = ml_dtypes.bfloat16


def build_nc():
    nc = bacc.Bacc(None)

    xT = nc.declare_dram_parameter("xT", [D, T], BF16, isOutput=False)
    cT = nc.declare_dram_parameter("cT", [D, T], BF16, isOutput=False)
    wq = nc.declare_dram_parameter("wq", [D, CS], BF16, isOutput=False)
    wks = nc.declare_dram_parameter("wks", [D, CS], BF16, isOutput=False)
    wkc = nc.declare_dram_parameter("wkc", [D, CS], BF16, isOutput=False)
    wvs = nc.declare_dram_parameter("wvs", [D, CS], BF16, isOutput=False)
    wvc = nc.declare_dram_parameter("wvc", [D, CS], BF16, isOutput=False)
    bq = nc.declare_dram_parameter("bq", [CS, 1], F32, isOutput=False)
    bks = nc.declare_dram_parameter("bks", [CS, 1], F32, isOutput=False)
    bkc = nc.declare_dram_parameter("bkc", [CS, 1], F32, isOutput=False)
    bvs = nc.declare_dram_parameter("bvs", [1, CS], F32, isOutput=False)
    bvc = nc.declare_dram_parameter("bvc", [1, CS], F32, isOutput=False)
    wo = nc.declare_dram_parameter("wo", [CS, D], BF16, isOutput=False)
    out = nc.declare_dram_parameter("out", [T, D], BF16, isOutput=True)

    with tile.TileContext(nc) as tc:
        _emit(nc, tc, xT, cT, wq, wks, wkc, wvs, wvc,
              bq, bks, bkc, bvs, bvc, wo, out)
    nc.compile()
    return nc


def _emit(nc, tc, xT, cT, wq, wks, wkc, wvs, wvc, bq, bks, bkc, bvs, bvc,
          wo, out):
    from contextlib import ExitStack

    ctx = ExitStack()
    with ctx:
        consts = ctx.enter_context(tc.tile_pool(name="consts", bufs=1))
        wpool = ctx.enter_context(tc.tile_pool(name="wpool", bufs=1))
        io_pool = ctx.enter_context(tc.tile_pool(name="io", bufs=64))
        qt_pool = ctx.enter_context(tc.tile_pool(name="qt", bufs=1))
        kt_pool = ctx.enter_context(tc.tile_pool(name="kt", bufs=1))
        v_pool = ctx.enter_context(tc.tile_pool(name="v", bufs=1))
        p_pool = ctx.enter_context(tc.tile_pool(name="p", bufs=16))
        outt_pool = ctx.enter_context(tc.tile_pool(name="outt", bufs=1))
        stage_pool = ctx.enter_context(tc.tile_pool(name="stage", bufs=3))
        misc_pool = ctx.enter_context(tc.tile_pool(name="misc", bufs=4))
        acc_pool = ctx.enter_context(tc.tile_pool(name="acc", bufs=2))
        an_pool = ctx.enter_context(tc.tile_pool(name="an", bufs=8))
        # PSUM: shared(2) + scores(4) + pv(2) = 8 banks
        ps_shared = ctx.enter_context(
            tc.tile_pool(name="ps_shared", bufs=2, space="PSUM"))
        ps_scores = ctx.enter_context(
            tc.tile_pool(name="ps_scores", bufs=2, space="PSUM"))
        ps_pv = ctx.enter_context(
            tc.tile_pool(name="ps_pv", bufs=2, space="PSUM"))

        # ---- small constants (cheap DMAs, emitted up front) ----
        b_sb = {}
        for name, b in (("bq", bq), ("bks", bks), ("bkc", bkc)):
            t = consts.tile([128, 2], F32, tag=f"b_{name}", name=f"b_{name}")
            nc.sync.dma_start(out=t, in_=b.rearrange("(a p) o -> p (a o)", p=128))
            b_sb[name] = t
        bv_sb = {}
        for name, b in (("bvs", bvs), ("bvc", bvc)):
            t = consts.tile([128, CS], F32, tag=f"bv_{name}", name=f"bv_{name}")
            nc.sync.dma_start(out=t, in_=b[:, :].to_broadcast([128, CS]))
            bv_sb[name] = t

        # weight tiles: each gets its own slot (bf16 keeps SBUF cheap)
        w_sb = {}
        for name, w in (("wq", wq), ("wks", wks), ("wvs", wvs),
                        ("wkc", wkc), ("wvc", wvc)):
            w_sb[name] = wpool.tile([128, DC, CS], BF16, tag=f"w_{name}",
                                    name=f"w_{name}")
        wo_sb = consts.tile([128, 2, D], BF16, tag="wo")

        qT_sb = [qt_pool.tile([128, T], BF16, tag=f"qT{cc}", name=f"qT{cc}")
                 for cc in range(2)]
        kT_sb = [kt_pool.tile([128, TK], BF16, tag=f"kT{cc}", name=f"kT{cc}")
                 for cc in range(2)]
        v_sb = [v_pool.tile([128, NH * (HD + 1)], BF16, tag=f"v{kc}",
                            name=f"v{kc}")
                for kc in range(32)]
        outT_sb = [outt_pool.tile([128, T], BF16, tag=f"outT{cc}",
                                  name=f"outT{cc}")
                   for cc in range(2)]

        # ---- deadline-scheduled work ----------------------------------
        # sched[(bi, kc)] = closures that MUST be emitted right after the
        # exp of iteration (block bi, k-chunk kc); kc == -1 means at block
        # start, before its first QK.  `pending` holds order-only work
        # (epilogues, out-projection) popped one per iteration when no
        # deadline work is due.
        sched = defaultdict(list)
        pending = deque()

        srcs = ((xT, "wks", "wvs", "bks", "bvs"),
                (cT, "wkc", "wvc", "bkc", "bvc"))
        waves = [[None] * DC for _ in range(8)]  # 8 waves of 8 [128,512] tiles
        chain_ps = {}

        def io_dma(w):
            src = srcs[w // 4][0]
            tc4 = w % 4

            def go():
                # the first wave gates the first exp: spread its transfers
                # over all three HWDGE issue queues
                engs = ((nc.sync, nc.scalar) if w == 0
                        else (nc.sync,))
                for dc in range(DC):
                    t = io_pool.tile([128, 512], BF16, tag="io",
                                     name=f"io_{w}_{dc}")
                    engs[dc % len(engs)].dma_start(
                        out=t,
                        in_=src[dc * 128:(dc + 1) * 128,
                                tc4 * 512:(tc4 + 1) * 512])
                    waves[w][dc] = t

            go._tag = f"io_{w}"
            return go

        def w_dma(name, w, eng=None):
            def go():
                (eng or nc.sync).dma_start(
                    out=w_sb[name], in_=w.rearrange("(a p) c -> p a c", p=128))

            return go

        def wo_dma():
            nc.sync.dma_start(
                out=wo_sb, in_=wo.rearrange("(a p) f -> p a f", p=128))

        def ckproj(w, cc, half, dst, coff, wn, bn):
            # half a [c,t]-projection chain (4 of 8 contraction steps);
            # halves share one PSUM accumulation group via chain_ps
            tc4 = w % 4

            def go():
                key = ("ck", w, cc, wn)
                if half == 0:
                    chain_ps[key] = ps_shared.tile(
                        [128, 512], F32, tag="ps", name=f"ps_ck_{w}_{cc}_{wn}")
                ps = chain_ps[key]
                for dc in range(half * 4, half * 4 + 4):
                    nc.tensor.matmul(
                        ps, (w_sb[wn][:, dc, cc * 128:(cc + 1) * 128]),
                        (waves[w][dc]),
                        start=(dc == 0), stop=(dc == DC - 1))
                if half == 1:
                    nc.vector.tensor_scalar_add(
                        dst[cc][:, coff + tc4 * 512:coff + (tc4 + 1) * 512],
                        ps, b_sb[bn][:, cc:cc + 1])

            go._tag = ("q0" if wn == "wq" else "k0") if (w == 0 and cc == 0) else ""
            go._half = half
            return go

        def vproj(w, sub, pair, half, wv_n, bv_n):
            # half a [t,c]-projection chain for one head pair (128 cols)
            kc = (w // 4) * 16 + (w % 4) * 4 + sub

            def go():
                key = ("v", w, sub, pair)
                if half == 0:
                    chain_ps[key] = ps_shared.tile(
                        [128, 512], F32, tag="ps", name=f"ps_v_{w}_{sub}_{pair}")
                ps = chain_ps[key]
                for dc in range(half * 4, half * 4 + 4):
                    nc.tensor.matmul(
                        ps[:, 0:128],
                        (waves[w][dc][:, sub * 128:(sub + 1) * 128]),
                        (w_sb[wv_n][:, dc, pair * 128:(pair + 1) * 128]),
                        start=(dc == 0), stop=(dc == DC - 1))
                if half == 1:
                    vt = v_sb[kc]
                    vt_v = vt[:].rearrange("p (h x) -> p h x", h=NH)
                    nc.vector.tensor_add(
                        vt_v[:, 2 * pair:2 * pair + 2, 0:HD],
                        ps[:, 0:128].rearrange("p (h x) -> p h x", h=2),
                        bv_sb[bv_n][:, pair * 128:(pair + 1) * 128]
                        .rearrange("p (h x) -> p h x", h=2))
                    nc.vector.memset(
                        vt_v[:, 2 * pair:2 * pair + 2, HD:HD + 1]
                        .rearrange("p h one -> p (h one)"), 1.0)

            go._tag = "v0" if (w == 0 and sub == 0 and pair == 0) else ""
            go._half = half
            return go

        # deadline assignment. blocks: bi = qc*2 + pair, 32 k-iterations
        # each.  An item due at (bi, kc) is emitted after exp(kc) and
        # before PV(kc-1) of that iteration.
        for w in range(8):
            src, wk_n, wv_n, bk_n, bv_n = srcs[w // 4]
            kc0 = (w // 4) * 16 + (w % 4) * 4  # first k-chunk of this wave
            # io prefetch ~6 iterations ahead of first use
            sched[(0, max(kc0 - 7, -1) if w else -1)].append(io_dma(w))
            # kT chains: cc=0 feeds block 0, cc=1 feeds block 1
            for cc in range(2):
                bi = cc
                due = max(kc0 - 2, -1) if (w or cc) else -1
                sched[(bi, due)].append(
                    ckproj(w, cc, 0, kT_sb, (w // 4) * T, wk_n, bk_n))
                sched[(bi, max(kc0 - 1, -1) if (w or cc) else -1)].append(
                    ckproj(w, cc, 1, kT_sb, (w // 4) * T, wk_n, bk_n))
            # v chains: pair p feeds block p, chunk kc0+sub needed by
            # PV at iteration kc0+sub+1
            for sub in range(4):
                for pair in range(2):
                    bi = pair
                    due = kc0 + sub - 1 if (w or sub or pair) else -1
                    sched[(bi, max(due, -1))].append(
                        vproj(w, sub, pair, 0, wv_n, bv_n))
                    sched[(bi, max(kc0 + sub, -1))].append(
                        vproj(w, sub, pair, 1, wv_n, bv_n))
        # q chains: qT[cc] block tc4 feeds block (tc4*2 + cc); each is
        # emitted in an earlier block (x-waves stay resident in SBUF)
        qdue = {(0, 0): ((0, -1), (0, -1)),
                (0, 1): ((0, 21), (0, 23)),
                (1, 0): ((1, 6), (1, 8)),
                (1, 1): ((1, 12), (1, 14)),
                (2, 0): ((2, 2), (2, 4)),
                (2, 1): ((3, 2), (3, 4)),
                (3, 0): ((4, 2), (4, 4)),
                (3, 1): ((5, 2), (5, 4))}
        for (tc4, cc), (due0, due1) in qdue.items():
            sched[due0].append(ckproj(tc4, cc, 0, qT_sb, 0, "wq", "bq"))
            sched[due1].append(ckproj(tc4, cc, 1, qT_sb, 0, "wq", "bq"))
        # weight DMAs: x-side + wq at the very start, ctx-side + wo a bit in
        wq_c = w_dma("wq", wq)
        wks_c = w_dma("wks", wks, nc.scalar)
        wvs_c = w_dma("wvs", wvs, nc.scalar)
        sched[(0, -1)].extend([wq_c, wks_c, wvs_c])
        sched[(0, 2)].insert(0, w_dma("wkc", wkc))
        sched[(0, 2)].insert(1, w_dma("wvc", wvc))
        sched[(0, 5)].insert(0, wo_dma)

        # prologue order: the chain gating the first exp goes first
        # (wq dma -> wave0 dma -> wks -> qproj -> kproj -> QK)
        first = {id(wq_c): 0, id(wks_c): 2, id(wvs_c): 7}
        for i, f in enumerate(sched[(0, -1)]):
            for nm, p in (("io_0", 1), ("q0", 3), ("k0", 5), ("v0", 8)):
                if getattr(f, "_tag", None) == nm:
                    first[id(f)] = p + getattr(f, "_half", 0)
        sched[(0, -1)].sort(key=lambda f: first.get(id(f), 50))

        # ---- out-projection closures -----------------------------------
        def make_outproj(qc):
            work = []
            for qt in range(qc * 4, qc * 4 + 4):
                qsl = slice(qt * 128, (qt + 1) * 128)
                for fc in range(2):
                    fsl = slice(fc * 512, (fc + 1) * 512)

                    def mm(qsl=qsl, fsl=fsl):
                        ps = ps_shared.tile([128, 512], F32, tag="ps",
                                            name="ps_op")
                        for cc in range(2):
                            nc.tensor.matmul(
                                ps, (outT_sb[cc][:, qsl]),
                                (wo_sb[:, cc, fsl]),
                                start=(cc == 0), stop=(cc == 1))
                        st = stage_pool.tile([128, 512], BF16, tag="stage",
                                             name="st_op")
                        nc.vector.tensor_copy(st, ps)
                        nc.sync.dma_start(out=out[qsl, fsl], in_=st)

                    work.append(mm)
            return work

        # ---- attention spine -------------------------------------------
        # flat 256-iteration pipeline over (qc, pair, kc).  Iteration g:
        #   exp(g) -> deadline pops -> QK(g+1)
        # QK runs a full iteration ahead of its exp so the activation
        # engine never waits on PE work.  PV is restructured: the exp
        # tiles (pt, a 16-deep ring) become the matmul STATIONARY operand
        # and the [V|ones] columns the 65-row moving operand, producing
        # [128q, 65] PSUM tiles -- 65 PE rows per (head, q-block, k-chunk)
        # instead of 512/2.  Accumulation runs in 8-k-chunk segments
        # (2 rotating PSUM banks, one open group at a time) flushed into an
        # SBUF fp32 accumulator; the softmax division is then a
        # per-partition reciprocal+scale on the DVE, and the [q, c]->[c, q]
        # transpose into outT is done by the DMA crossbar
        # (dma_start_transpose), costing no engine time at all.
        iters = [(qc, pair, kc)
                 for qc in range(4) for pair in range(2) for kc in range(32)]
        gsched = {}
        for (bi, kc), fs in sched.items():
            g = bi * 32 + kc if kc >= 0 else bi * 32 - 1
            gsched.setdefault(g, []).extend(fs)
        sched.clear()

        s2_of = {}
        pt_of = {}
        an_of = {}

        def emit_qk(g):
            qc, pair, kc = iters[g]
            qs = slice(qc * 512, (qc + 1) * 512)
            ks = slice(kc * 128, (kc + 1) * 128)
            s2 = ps_scores.tile([128, 1024], F32, tag="s", name=f"s2_{g}")
            nc.tensor.matmul(
                s2[:, 0:512], (kT_sb[pair][0:64, ks]),
                (qT_sb[pair][0:64, qs]), start=True, stop=True)
            nc.tensor.matmul(
                s2[:, 512:1024], (kT_sb[pair][64:128, ks]),
                (qT_sb[pair][64:128, qs]), start=True, stop=True,
                tile_position=(64, 0))
            s2_of[g] = s2

        def make_pv_seg(bi, pair, seg, h, qb, acc):
            def go():
                pv = ps_pv.tile([128, 512], F32, tag="pv",
                                name=f"pv_{bi}_{seg}_{h}_{qb}")
                for j in range(8):
                    kc = seg * 8 + j
                    pt = pt_of[bi * 32 + kc]
                    nc.tensor.matmul(
                        pv[:, 0:65],
                        (pt[:, h * 512 + qb * 128:h * 512 + (qb + 1) * 128]),
                        (v_sb[kc][:, (2 * pair + h) * 65:
                                  (2 * pair + h + 1) * 65]),
                        start=(j == 0), stop=(j == 7))
                dst = acc[:, (h * 4 + qb) * 65:(h * 4 + qb + 1) * 65]
                if seg == 0:
                    nc.vector.tensor_copy(dst, pv[:, 0:65])
                else:
                    nc.vector.tensor_add(dst, dst, pv[:, 0:65])

            return go

        def make_div(bi, qc, pair, h, qb, acc):
            def go():
                base = (h * 4 + qb) * 65
                if h == 0:
                    an_of[(bi, qb)] = an_pool.tile(
                        [128, 128], BF16, tag="an", name=f"an_{bi}_{qb}")
                an = an_of[(bi, qb)]
                r = misc_pool.tile([128, 1], F32, tag="r",
                                   name=f"r_{bi}_{h}_{qb}")
                nc.vector.reciprocal(r, acc[:, base + 64:base + 65])
                nc.vector.tensor_scalar_mul(
                    an[:, h * 64:(h + 1) * 64],
                    acc[:, base:base + 64], r)

            return go

        def make_tp(bi, qc, pair, qb):
            def go():
                nc.sync.dma_start_transpose(
                    out=outT_sb[pair][:, qc * 512 + qb * 128:
                                      qc * 512 + (qb + 1) * 128],
                    in_=an_of[(bi, qb)][:])

            return go

        for f in gsched.pop(-1, ()):
            f()
        emit_qk(0)
        acc = None
        for g, (qc, pair, kc) in enumerate(iters):
            bi = qc * 2 + pair
            if kc == 0:
                acc = acc_pool.tile([128, 8 * 65], F32, tag="acc",
                                    name=f"acc_{bi}")
            pt = p_pool.tile([128, 1024], BF16, tag="pt", name=f"pt{g}")
            nc.scalar.activation(pt, s2_of.pop(g), AFT.Exp)
            pt_of[g] = pt
            due = gsched.pop(g, None)
            if due:
                for f in due:
                    f()
            elif pending:
                pending.popleft()()
            if g + 1 < len(iters):
                emit_qk(g + 1)
            if kc % 8 == 7:
                # schedule this segment's 8 PV groups over the next 8 iters
                seg = kc // 8
                for i, (h, qb) in enumerate(
                        (h, qb) for h in range(2) for qb in range(4)):
                    gsched.setdefault(g + 1 + i, []).append(
                        make_pv_seg(bi, pair, seg, h, qb, acc))
            if kc == 31:
                # divisions after the last flush, transposes after those,
                # out-projection once both pairs of this qc are transposed
                for i, (h, qb) in enumerate(
                        (h, qb) for h in range(2) for qb in range(4)):
                    gsched.setdefault(g + 9 + i, []).append(
                        make_div(bi, qc, pair, h, qb, acc))
                for qb in range(4):
                    gsched.setdefault(g + 17 + qb, []).append(
                        make_tp(bi, qc, pair, qb))
                if pair == 1:
                    for i, fn in enumerate(make_outproj(qc)):
                        gsched.setdefault(g + 21 + i, []).append(fn)
        # drain: deadlines past the last iteration, in order
        for g in sorted(gsched):
            for f in gsched.pop(g):
                f()
        while pending:
            pending.popleft()()


_NC_CACHE = None


def kernel(**inputs):
    global _NC_CACHE
    if _NC_CACHE is None:
        _NC_CACHE = build_nc()
    nc = _NC_CACHE

    f = {k: np.asarray(v, dtype=np.float32) for k, v in inputs.items()}
    x, context = f["x"], f["context"]
    B = x.shape[0]

    xTs = [np.ascontiguousarray(x[b].T).astype(BF) for b in range(B)]
    cTs = [np.ascontiguousarray(context[b].T).astype(BF) for b in range(B)]

    in_maps = []
    for b in range(B):
        for hg in range(4):
            sl = slice(hg * CS, (hg + 1) * CS)
            in_maps.append({
                "xT": xTs[b],
                "cT": cTs[b],
                "wq": (np.ascontiguousarray(f["Wq"][:, sl]) * 0.125).astype(BF),
                "wks": np.ascontiguousarray(f["Wks"][:, sl]).astype(BF),
                "wkc": np.ascontiguousarray(f["Wkc"][:, sl]).astype(BF),
                "wvs": np.ascontiguousarray(f["Wvs"][:, sl]).astype(BF),
                "wvc": np.ascontiguousarray(f["Wvc"][:, sl]).astype(BF),
                "bq": (f["bq"][sl] * 0.125).reshape(CS, 1).copy(),
                "bks": f["bks"][sl].reshape(CS, 1).copy(),
                "bkc": f["bkc"][sl].reshape(CS, 1).copy(),
                "bvs": f["bvs"][sl].reshape(1, CS).copy(),
                "bvc": f["bvc"][sl].reshape(1, CS).copy(),
                "wo": np.ascontiguousarray(f["Wo"][sl, :]).astype(BF),
            })

    res = run_bass_kernel_spmd(nc, in_maps, list(range(N_CORES))).results

    bo = f["bo"]
    out = np.empty((B, T, D), dtype=np.float32)
    for b in range(B):
        acc = res[b * 4 + 0]["out"].astype(np.float32)
        for hg in range(1, 4):
            acc += res[b * 4 + hg]["out"].astype(np.float32)
        out[b] = acc + bo
    return out
